# revision 50
# baseline (speedup 1.0000x reference)
"""Bass/Trainium2 kernel for nn_DecoderAttention (gnn message passing).

Math: q = query @ WQ.T is scattered to the 64 global nodes (glob_idx) and is
zero everywhere else, and the output only reads out[glob_idx].  Therefore only
edges whose dst is a global node contribute to the result.  Host-side we
partition the edge list by dst (CSR sort, as the sharding hint prescribes) and
shard the 64 global nodes across the 8 cores (node list i::8 -> core i); the
per-core input shard is the <=128 x rows referenced by that core's edges,
re-laid-out host-side into one contiguous bf16 header block (one direct DMA,
no on-device gather).  Each core projects its gathered rows with K/V, does the
per-node masked softmax and aggregation, and applies the output projection for
its 8 rows; all tensor FLOPs of the module run on device in bf16 (tolerance
2e-2; measured rel err ~1e-3).

A general fallback using indirect row_ptr/src/x gathers handles arbitrary
glob_idx / caps that overflow the fast layout.
"""

import os

import numpy as np
import ml_dtypes

import concourse.bacc as bacc
import concourse.mybir as mybir
from concourse.bass import IndirectOffsetOnAxis
from concourse.bass_utils import run_bass_kernel_spmd
from concourse.tile import TileContext

BF16 = ml_dtypes.bfloat16


class _SlimTailTileContext(TileContext):
    """TileContext whose kernel tail skips the final all-engine barrier.

    The standard tail is drain -> barrier -> sem clears -> barrier.  The last
    barrier only isolates the clears from code following the TileContext in
    multi-kernel modules; this NEFF ends right after, and each engine halts
    only once its own instruction stream (including the clears) completes, so
    it is dead weight here."""

    def _drain_and_barrier(self, tick_clock, wait_clock):
        from concourse.tile import ScopedClock

        nc = self.nc
        drain_inst = nc.sync.drain()
        wait_clock.add_sem_waits(
            drain_inst.ins, ScopedClock({None: tick_clock.global_clock})
        )
        # One drain->sem hop orders the gpsimd sem clears after all work,
        # instead of the full (expensive) all-engine EVSEM butterfly.
        done = nc.alloc_semaphore("tail_done")
        drain_inst.then_inc(done, 1)
        nc.gpsimd.wait_ge(done, 1)
        assert self.sems is not None
        popped = nc._tile_sem_poison_stack.pop()
        assert popped is self._sem_poison
        # sem_clear only (skip clear_and_free's dma_reset: each NEFF load
        # re-initializes the DMA rings, and the reset machinery is the
        # dominant cost of the kernel tail)
        from concourse.bass import compact_to_ranges
        nums = sorted(s.num if hasattr(s, "num") else s
                      for s in list(self.sems.allocated().values()) + [done])
        for r in compact_to_ranges(nums):
            nc.gpsimd.sem_clear(r)

D = 256
H = 4
DK = 64
NV = 40000
NE = 320000
B = 64
NCORES = 8
P = 128
NPC = B // NCORES  # nodes (output rows) per core: 8

F32 = mybir.dt.float32
I32 = mybir.dt.int32
BF = mybir.dt.bfloat16

_cache: dict = {}
_gc_min_zero = [True]  # does some global node have zero incoming edges?

last_results = None  # BassKernelResults of the most recent run (for harness)

# fast-path hdr column layout (all bf16)
C_XSEL = 0                      # [:, 0:256]   gathered x rows, pre-transposed
                                #   host-side: hdr[d, t*128+p] = x_sel[p, t*128+d]
C_Q = C_XSEL + D                # [:, 256:272] query^T chunks (2 x [128, 8])
C_EXPJ = C_Q + 2 * NPC          # [:, 272:280] slot->node one-hot (lhsT)
C_NEGB = C_EXPJ + NPC           # [:, 280]     exp bias: (valid-1)*30
C_VALID = C_NEGB + 1            # [:, 281]     slot validity 0/1
C_MLO = C_VALID + 1             # [:, 282]     partition < 64 mask
C_MHI = C_MLO + 1               # [:, 283]     partition >= 64 mask
HC = C_MHI + 1                  # 284

# wall column layout (all bf16, shared across cores); each weight is a
# separate DMA spread over the three DMA-capable rings so they land in
# the order the compute chain consumes them (wq, wv, wo).  WK ships
# separately in natural (out-dim major) head-packed layout for the
# score-side fold (see CT below).
W_WQ = 0                        # [:, 0:512]      WQ.T d-chunks
W_WV = W_WQ + 2 * D             # [:, 512:1024]   WV.T d-chunks
W_WO = W_WV + 2 * D             # [:, 1024:1536]  WO.T d-chunks
WC = W_WO + 2 * D               # 1536


def _build_fast(guard: bool):
    """Fast-path SPMD program: direct-DMA inputs only, bf16 compute.

    Requires glob_idx == arange(64) and each core's <=128 relevant edges
    pre-gathered host-side into hdr (see kernel()).  guard=True adds the
    empty-node denominator guard (only needed when some global node has
    no incoming edges)."""
    nc = bacc.Bacc("TRN2", target_bir_lowering=False, debug=False,
                   num_devices=NCORES)

    hdr_d = nc.dram_tensor("hdr", [P, HC], BF, kind="ExternalInput")
    wall_d = nc.dram_tensor("wall", [P, WC], BF, kind="ExternalInput")
    wkn_d = nc.dram_tensor("wkn", [P, 2 * D], BF, kind="ExternalInput")
    # output is r^T: out_r[d, t*8+j] = r[j, t*128+d]
    out_d = nc.dram_tensor("out_r", [P, 2 * NPC], F32, kind="ExternalOutput")

    NAGG = D + H + 1 if guard else D + H

    with _SlimTailTileContext(nc) as tc:
        with (
            tc.tile_pool(name="sbuf", bufs=1) as sb,
            tc.tile_pool(name="psum", bufs=1, space="PSUM") as pp,
            tc.tile_pool(name="psmall", bufs=2, space="PSUM") as ps,
        ):
            hdr = sb.tile([P, HC], BF, tag="hdr")
            nc.sync.dma_start(out=hdr[:], in_=hdr_d[:])
            wall = sb.tile([P, WC], BF, tag="wall")
            wkn = sb.tile([P, 2 * D], BF, tag="wkn")
            # wq in t-halves: the t=0 accumulation of qmT fires on the
            # first 64KB instead of waiting for the full transfer
            nc.scalar.dma_start(out=wall[:, W_WQ:W_WQ + D],
                                in_=wall_d[:, W_WQ:W_WQ + D])
            nc.scalar.dma_start(out=wall[:, W_WQ + D:W_WQ + 2 * D],
                                in_=wall_d[:, W_WQ + D:W_WQ + 2 * D])
            nc.scalar.dma_start(out=wkn[:], in_=wkn_d[:])
            nc.gpsimd.dma_start(out=wall[:, W_WV:W_WV + 2 * D],
                                in_=wall_d[:, W_WV:W_WV + 2 * D])
            nc.sync.dma_start(out=wall[:, W_WO:W_WO + 2 * D],
                              in_=wall_d[:, W_WO:W_WO + 2 * D])
            # identity built on-chip (gpsimd finishes before the DMA rings
            # even come up), keeping the DMA window for real payload
            ident = sb.tile([P, P], BF, tag="ident")
            from concourse.masks import make_identity
            make_identity(nc, ident[:])

            ej = hdr[:, C_EXPJ:C_EXPJ + NPC]

            # exp bias to f32 (activation bias operand)
            negb = sb.tile([P, 1], F32, tag="negb")
            nc.vector.tensor_copy(out=negb[:], in_=hdr[:, C_NEGB:C_NEGB + 1])

            # x_sel^T arrives pre-transposed in the hdr (host layout)
            xt = hdr[:, C_XSEL:C_XSEL + D]

            # qmT[hk, j] = (query_mine @ WQ.T)^T, computed directly in
            # transposed layout (hk on partitions) -- no PE transposes
            qmt_ps = ps.tile([P, 2 * NPC], F32, tag="ps_small")
            for u in range(2):
                for t in range(2):
                    nc.tensor.matmul(
                        out=qmt_ps[:, u * NPC:(u + 1) * NPC],
                        lhsT=wall[:, W_WQ + t * D + u * P:
                                  W_WQ + t * D + (u + 1) * P],
                        rhs=hdr[:, C_Q + t * NPC:C_Q + (t + 1) * NPC],
                        start=(t == 0), stop=(t == 1))
            # zero-pad qmT per head half so each 128-deep contraction chunk
            # only sees its own head's 64 rows: qmtp cols (u, hl, j).
            # Masked straight out of PSUM, split across vector and gpsimd.
            hmask = sb.tile([P, 2], F32, tag="hmask")
            nc.vector.tensor_copy(out=hmask[:], in_=hdr[:, C_MLO:C_MHI + 1])
            qmtp = sb.tile([P, 4 * NPC], BF, tag="qmtp")
            for u in range(2):
                for hl in range(2):
                    nc.vector.tensor_scalar(
                        out=qmtp[:, (u * 2 + hl) * NPC:
                                 (u * 2 + hl + 1) * NPC],
                        in0=qmt_ps[:, u * NPC:(u + 1) * NPC],
                        scalar1=hmask[:, hl:hl + 1], scalar2=None,
                        op0=mybir.AluOpType.mult)

            # CT[d, (h,j)] = sum_k WK[h*64+k, d] * qm[j, h*64+k]: the k-proj
            # folded into the query side, so scores are one small matmul on
            # the gathered rows instead of a full K projection.  wkn holds
            # WK natural 128-row chunks; head separation comes from qmtp.
            ct_ps = pp.tile([P, 2 * H * NPC], F32, tag="ps_ct")
            for t in range(2):
                for u in range(2):
                    nc.tensor.matmul(
                        out=ct_ps[:, t * H * NPC + u * 2 * NPC:
                                  t * H * NPC + (u + 1) * 2 * NPC],
                        lhsT=wkn[:, u * D + t * P:u * D + (t + 1) * P],
                        rhs=qmtp[:, u * 2 * NPC:(u + 1) * 2 * NPC],
                        start=True, stop=True)
            ct = sb.tile([P, 2 * H * NPC], BF, tag="ct")
            nc.vector.tensor_copy(out=ct[:], in_=ct_ps[:])

            # per-slot scores for all (head, node) pairs, then select own node
            s_ps = ps.tile([P, H * NPC], F32, tag="ps_small")
            for t in range(2):
                nc.tensor.matmul(out=s_ps[:], lhsT=xt[:, t * P:(t + 1) * P],
                                 rhs=ct[:, t * H * NPC:(t + 1) * H * NPC],
                                 start=(t == 0), stop=(t == 1))
            sm = sb.tile([P, H * NPC], F32, tag="sm")
            nc.vector.tensor_tensor(
                out=sm[:].rearrange("p (h j) -> p h j", h=H),
                in0=s_ps[:].rearrange("p (h j) -> p h j", h=H),
                in1=ej.rearrange("p (o j) -> p o j", o=1)
                    .to_broadcast([P, H, NPC]),
                op=mybir.AluOpType.mult)
            s = sb.tile([P, H], F32, tag="s")
            nc.vector.tensor_reduce(
                out=s[:], in_=sm[:].rearrange("p (h j) -> p h j", h=H),
                axis=mybir.AxisListType.X, op=mybir.AluOpType.add)

            # V projection of the gathered rows
            v_ps = pp.tile([P, D], F32, tag="ps_v")
            for t in range(2):
                nc.tensor.matmul(out=v_ps[:], lhsT=xt[:, t * P:(t + 1) * P],
                                 rhs=wall[:, W_WV + t * D:W_WV + (t + 1) * D],
                                 start=(t == 0), stop=(t == 1))
            # agg = [e-weighted v | e (| valid)]  (bf16 so the reduction
            # matmul runs at full PE rate; accumulation is f32 in PSUM)
            agg = sb.tile([P, NAGG], BF, tag="agg")
            nc.scalar.activation(out=agg[:, D:D + H], in_=s[:],
                                 func=mybir.ActivationFunctionType.Exp,
                                 bias=negb[:],
                                 scale=float(1.0 / np.sqrt(DK)))
            if guard:
                nc.vector.tensor_copy(out=agg[:, D + H:D + H + 1],
                                      in_=hdr[:, C_VALID:C_VALID + 1])
            nc.vector.tensor_tensor(
                out=agg[:, 0:D].rearrange("p (h d) -> p h d", h=H),
                in0=v_ps[:].rearrange("p (h d) -> p h d", h=H),
                in1=agg[:, D:D + H].to_broadcast([P, H, DK]),
                op=mybir.AluOpType.mult)

            # per-node reduction: [numer | denom (| count)]
            acc_ps = ps.tile([NPC, NAGG], F32, tag="ps_small")
            nc.tensor.matmul(out=acc_ps[:], lhsT=ej, rhs=agg[:],
                             start=True, stop=True)

            rec = sb.tile([NPC, H], F32, tag="rec")
            if guard:
                # guard empty nodes: denom += (count == 0)
                iszero = sb.tile([NPC, 1], F32, tag="iszero")
                nc.vector.tensor_scalar(out=iszero[:],
                                        in0=acc_ps[:, D + H:D + H + 1],
                                        scalar1=0.5, scalar2=None,
                                        op0=mybir.AluOpType.is_lt)
                den = sb.tile([NPC, H], F32, tag="den")
                nc.vector.tensor_scalar(out=den[:], in0=acc_ps[:, D:D + H],
                                        scalar1=iszero[:], scalar2=None,
                                        op0=mybir.AluOpType.add)
                nc.vector.reciprocal(out=rec[:], in_=den[:])
            else:
                nc.vector.reciprocal(out=rec[:], in_=acc_ps[:, D:D + H])
            onode = sb.tile([NPC, D], BF, tag="onode")
            nc.vector.tensor_tensor(
                out=onode[:].rearrange("p (h d) -> p h d", h=H),
                in0=acc_ps[:, 0:D].rearrange("p (h d) -> p h d", h=H),
                in1=rec[:].to_broadcast([NPC, H, DK]),
                op=mybir.AluOpType.mult)

            # r = out_node @ WO.T
            ot_ps = ps.tile([P, 2 * NPC], BF, tag="ps_small")
            for t in range(2):
                nc.tensor.transpose(out=ot_ps[:, t * NPC:(t + 1) * NPC],
                                    in_=onode[:, t * P:(t + 1) * P],
                                    identity=ident[0:NPC, 0:NPC])
            ot = sb.tile([P, 2 * NPC], BF, tag="ot")
            nc.vector.tensor_copy(out=ot[:], in_=ot_ps[:])
            # r^T directly (full-partition copies and a tiny out DMA):
            # rT[d, (t,j)] = r[j, t*128+d] = sum_u WO[t*128+d, u*128+d'] ...
            r_ps = ps.tile([P, 2 * NPC], F32, tag="ps_small")
            for t in range(2):
                for u in range(2):
                    nc.tensor.matmul(
                        out=r_ps[:, t * NPC:(t + 1) * NPC],
                        lhsT=wall[:, W_WO + u * D + t * P:
                                  W_WO + u * D + (t + 1) * P],
                        rhs=ot[:, u * NPC:(u + 1) * NPC],
                        start=(u == 0), stop=(u == 1))
            r_sb = sb.tile([P, 2 * NPC], F32, tag="r_sb")
            nc.vector.tensor_copy(out=r_sb[:], in_=r_ps[:])
            nc.sync.dma_start(out=out_d[:], in_=r_sb[:])

    nc.compile()
    return nc


def kernel(query, x, WQ, WK, WV, WO, src, dst, glob_idx):
    global last_results
    query = np.ascontiguousarray(np.asarray(query, dtype=np.float32))
    x = np.ascontiguousarray(np.asarray(x, dtype=np.float32))
    src32 = np.asarray(src, dtype=np.int32)
    dst32 = np.asarray(dst, dtype=np.int32)
    glob = np.asarray(glob_idx, dtype=np.int32)
    WQ = np.asarray(WQ, np.float32)
    WK = np.asarray(WK, np.float32)
    WV = np.asarray(WV, np.float32)
    WO = np.asarray(WO, np.float32)

    # partition (CSR-sort) edge list by dst shard (dst % 8), then dst
    shard = dst32 % NCORES
    order = np.lexsort((dst32, shard))
    s_src = src32[order]
    s_dst = dst32[order]
    s_shard = shard[order]
    shard_start = np.searchsorted(s_shard, np.arange(NCORES + 1))

    # per-global-node edge counts (for capacity + fast-path check)
    rel = dst32 < B
    gc = np.bincount(dst32[rel], minlength=B) if rel.any() else \
        np.zeros(B, np.int64)

    cap16_ok = gc.max() <= 16 if len(gc) else True
    pref_ok = all(gc[c::NCORES].sum() <= P for c in range(NCORES))
    _gc_min_zero[0] = bool(gc.min() == 0) if len(gc) else True
    fast = (np.array_equal(glob, np.arange(B, dtype=glob.dtype))
            and cap16_ok and pref_ok
            and not bool(int(os.environ.get("BASSK_FORCE_GENERAL", "0"))))

    if fast:
        res = _run_fast(query, x, s_src, s_dst, shard_start, WQ, WK, WV, WO)
    else:
        perm = np.argsort(dst32, kind="stable")
        sorted_src = np.ascontiguousarray(src32[perm])
        sorted_dst = dst32[perm]
        row_ptr = np.searchsorted(sorted_dst,
                                  np.arange(NV + 1)).astype(np.int32)
        gcnt = int((row_ptr[glob + 1] - row_ptr[glob]).max()) if len(glob) \
            else 0
        cap = 16
        while cap < gcnt:
            cap *= 2
        res = _run_general(query, x, sorted_src, row_ptr, glob, cap,
                           WQ, WK, WV, WO)
    last_results = res
    if fast:
        # per-core out is r^T [128, (t, j)]: r_c[j, t*128+d] = out[d, t*8+j]
        outs = [np.transpose(
            np.asarray(res.results[c]["out_r"]).reshape(P, 2, NPC),
            (2, 1, 0)).reshape(NPC, D) for c in range(NCORES)]
    else:
        outs = [res.results[c]["out_r"] for c in range(NCORES)]
    return np.ascontiguousarray(
        np.stack(outs, axis=1).reshape(B, D).astype(np.float32))


def _run_fast(query, x, s_src, s_dst, shard_start, WQ, WK, WV, WO):
    cap = 16
    guard = bool(_gc_min_zero[0])

    # weight wall (shared): W^T d-chunks, bf16
    wall = np.zeros((P, WC), np.float32)
    for t in range(2):
        dd = slice(t * P, (t + 1) * P)
        wall[:, W_WQ + t * D:W_WQ + (t + 1) * D] = WQ.T[dd]
        wall[:, W_WV + t * D:W_WV + (t + 1) * D] = WV.T[dd]
        wall[:, W_WO + t * D:W_WO + (t + 1) * D] = WO.T[dd]
    wall_bf = np.ascontiguousarray(wall.astype(BF16))
    # WK natural 128-row chunks side by side: wkn[p, u*D + d] = WK[u*128+p, d]
    wkn = np.ascontiguousarray(
        np.concatenate([WK[0:P, :], WK[P:2 * P, :]], axis=1).astype(BF16))

    nos = np.arange(P) // cap
    expj = np.zeros((P, NPC), np.float32)
    expj[np.arange(P), nos] = 1.0

    qT = query.T  # (D, B)
    in_maps = []
    for c in range(NCORES):
        lo, hi = int(shard_start[c]), int(shard_start[c + 1])
        sh_dst = s_dst[lo:hi]
        sh_src = s_src[lo:hi]
        n = hi - lo
        # shard-local row_ptr over my 8 nodes (c, c+8, .., c+56) + end
        my_nodes = c + NCORES * np.arange(NPC + 1)  # node c+64 bounds the end
        rp9 = np.searchsorted(sh_dst, my_nodes).astype(np.int64)
        offs_col = rp9[nos] + np.arange(P) % cap
        valid_col = (offs_col < rp9[nos + 1]).astype(np.float32)
        if n > 0:
            slot_src = np.where(offs_col < n,
                                sh_src[np.minimum(offs_col, n - 1)], 0)
        else:
            slot_src = np.zeros(P, np.int64)
        hdr = np.zeros((P, HC), np.float32)
        xs = x[slot_src]  # [128 slots, 256]; ship transposed per d-chunk
        for t in range(2):
            hdr[:, C_XSEL + t * P:C_XSEL + (t + 1) * P] = \
                xs[:, t * P:(t + 1) * P].T
        for t in range(2):
            hdr[:, C_Q + t * NPC:C_Q + (t + 1) * NPC] = \
                qT[t * P:(t + 1) * P, c::NCORES]
        hdr[:, C_EXPJ:C_EXPJ + NPC] = expj
        hdr[:, C_NEGB] = (valid_col - 1.0) * 30.0
        hdr[:, C_VALID] = valid_col
        hdr[:, C_MLO] = (np.arange(P) < DK).astype(np.float32)
        hdr[:, C_MHI] = (np.arange(P) >= DK).astype(np.float32)
        in_maps.append(dict(wall=wall_bf, wkn=wkn,
                            hdr=np.ascontiguousarray(hdr.astype(BF16))))

    key = ("fastbf", guard)
    if key not in _cache:
        _cache[key] = _build_fast(guard)
    nc = _cache[key]

    trace = bool(int(os.environ.get("BASSK_TRACE", "0")))
    return run_bass_kernel_spmd(nc, in_maps, core_ids=list(range(NCORES)),
                                trace=trace)


# ---------------------------------------------------------------------------
# general fallback (from validated v1 program)
# ---------------------------------------------------------------------------

def _expanders(cap):
    nslots = NPC * cap
    nch = nslots // P
    npc_chunk = P // cap
    expjt = np.zeros((NPC, P * nch), np.float32)
    expj = np.zeros((P, NPC * nch), np.float32)
    for k in range(nch):
        j_of_p = np.arange(P) // cap + k * npc_chunk
        expjt[j_of_p, k * P + np.arange(P)] = 1.0
        expj[np.arange(P), k * NPC + j_of_p] = 1.0
    woff = (np.arange(P) % cap).astype(np.float32)
    return expjt, expj, woff, nch


def _build_general(cap: int):
    """Build the SPMD Bass program. cap = edge slots per node (power of two,
    NPC*cap multiple of 128)."""
    nslots = NPC * cap
    n_chunks = nslots // P
    assert nslots % P == 0
    npc_chunk = P // cap  # nodes per 128-slot chunk

    nc = bacc.Bacc("TRN2", target_bir_lowering=False, debug=False,
                   num_devices=NCORES)

    # ---- DRAM I/O ----
    x_d = nc.dram_tensor("x", [NV, D], F32, kind="ExternalInput")
    srcs_d = nc.dram_tensor("srcs", [NE + cap, 1], I32, kind="ExternalInput")
    rp_d = nc.dram_tensor("row_ptr", [NV + 1, 1], I32, kind="ExternalInput")
    qy_d = nc.dram_tensor("query", [B, D], F32, kind="ExternalInput")
    wqt_d = nc.dram_tensor("wqt", [D, D], F32, kind="ExternalInput")
    wkt_d = nc.dram_tensor("wkt", [D, D], F32, kind="ExternalInput")
    wvt_d = nc.dram_tensor("wvt", [D, D], F32, kind="ExternalInput")
    wot_d = nc.dram_tensor("wot", [D, D], F32, kind="ExternalInput")
    sel_d = nc.dram_tensor("sel", [B, NPC], F32, kind="ExternalInput")
    expjt_d = nc.dram_tensor("expjt", [NPC, P * n_chunks], F32,
                             kind="ExternalInput")
    expj_d = nc.dram_tensor("expj", [P, NPC * n_chunks], F32,
                            kind="ExternalInput")
    woff_d = nc.dram_tensor("win_off", [P, 1], F32, kind="ExternalInput")
    ident_d = nc.dram_tensor("ident", [P, P], F32, kind="ExternalInput")
    mgs_d = nc.dram_tensor("my_glob_s", [NPC, 1], I32, kind="ExternalInput")
    mge_d = nc.dram_tensor("my_glob_e", [NPC, 1], I32, kind="ExternalInput")
    out_d = nc.dram_tensor("out_r", [NPC, D], F32, kind="ExternalOutput")

    with _SlimTailTileContext(nc) as tc:
        with (
            tc.tile_pool(name="const", bufs=1) as cpool,
            tc.tile_pool(name="work", bufs=1) as wpool,
            tc.tile_pool(name="psum", bufs=1, space="PSUM") as ppool,
            tc.tile_pool(name="psum_small", bufs=2, space="PSUM") as spool,
        ):
            # ---- constant / weight loads (issued early, overlap the chain) --
            qy = cpool.tile([B, D], F32, tag="qy")
            nc.sync.dma_start(out=qy[:], in_=qy_d[:])
            wq = cpool.tile([P, 2 * D], F32, tag="wq")  # [d-chunk t] at cols t*D
            wk = cpool.tile([P, 2 * D], F32, tag="wk")
            wv = cpool.tile([P, 2 * D], F32, tag="wv")
            wo = cpool.tile([P, 2 * D], F32, tag="wo")
            for t in range(2):
                nc.sync.dma_start(out=wq[:, t * D:(t + 1) * D],
                                  in_=wqt_d[t * P:(t + 1) * P, :])
                nc.sync.dma_start(out=wk[:, t * D:(t + 1) * D],
                                  in_=wkt_d[t * P:(t + 1) * P, :])
                nc.sync.dma_start(out=wv[:, t * D:(t + 1) * D],
                                  in_=wvt_d[t * P:(t + 1) * P, :])
                nc.sync.dma_start(out=wo[:, t * D:(t + 1) * D],
                                  in_=wot_d[t * P:(t + 1) * P, :])
            sel = cpool.tile([B, NPC], F32, tag="sel")
            nc.sync.dma_start(out=sel[:], in_=sel_d[:])
            expjt = cpool.tile([NPC, P * n_chunks], F32, tag="expjt")
            nc.sync.dma_start(out=expjt[:], in_=expjt_d[:])
            expj = cpool.tile([P, NPC * n_chunks], F32, tag="expj")
            nc.sync.dma_start(out=expj[:], in_=expj_d[:])
            woff = cpool.tile([P, 1], F32, tag="woff")
            nc.sync.dma_start(out=woff[:], in_=woff_d[:])
            ident = cpool.tile([P, P], F32, tag="ident")
            nc.sync.dma_start(out=ident[:], in_=ident_d[:])
            mgs = cpool.tile([NPC, 1], I32, tag="mgs")
            nc.sync.dma_start(out=mgs[:], in_=mgs_d[:])
            mge = cpool.tile([NPC, 1], I32, tag="mge")
            nc.sync.dma_start(out=mge[:], in_=mge_d[:])

            # ---- row_ptr[glob] and row_ptr[glob+1] (one indirect gather) ----
            st_i = wpool.tile([NPC, 1], I32, tag="st_i")
            nc.gpsimd.indirect_dma_start(
                out=st_i[:], out_offset=None, in_=rp_d[:],
                in_offset=IndirectOffsetOnAxis(ap=mgs[:], axis=0))
            en_i = wpool.tile([NPC, 1], I32, tag="en_i")
            nc.gpsimd.indirect_dma_start(
                out=en_i[:], out_offset=None, in_=rp_d[:],
                in_offset=IndirectOffsetOnAxis(ap=mge[:], axis=0))
            st_f = wpool.tile([NPC, 1], F32, tag="st_f")
            nc.vector.tensor_copy(out=st_f[:], in_=st_i[:])
            en_f = wpool.tile([NPC, 1], F32, tag="en_f")
            nc.vector.tensor_copy(out=en_f[:], in_=en_i[:])

            # ---- q_glob = query @ WQ.T ; q_mine = my 8 rows ----
            qyt = wpool.tile([P, 2 * B], F32, tag="qyt")  # query^T d-chunks
            for t in range(2):
                pt = spool.tile([P, B], F32, tag="ps_small")
                nc.tensor.transpose(out=pt[:], in_=qy[:, t * P:(t + 1) * P],
                                    identity=ident[:B, :B])
                nc.vector.tensor_copy(out=qyt[:, t * B:(t + 1) * B], in_=pt[:])
            qg_ps = ppool.tile([B, D], F32, tag="ps_qg")
            for t in range(2):
                nc.tensor.matmul(out=qg_ps[:], lhsT=qyt[:, t * B:(t + 1) * B],
                                 rhs=wq[:, t * D:(t + 1) * D],
                                 start=(t == 0), stop=(t == 1))
            qg = wpool.tile([B, D], F32, tag="qg")
            nc.vector.tensor_copy(out=qg[:], in_=qg_ps[:])
            qm_ps = spool.tile([NPC, D], F32, tag="ps_small")
            nc.tensor.matmul(out=qm_ps[:], lhsT=sel[:], rhs=qg[:],
                             start=True, stop=True)
            qm = wpool.tile([NPC, D], F32, tag="qm")
            nc.vector.tensor_copy(out=qm[:], in_=qm_ps[:])

            # ---- accumulator over chunks (numer | denom | count) ----
            acc = wpool.tile([NPC, D + H + 1], F32, tag="acc")

            for k in range(n_chunks):
                ejt = expjt[:, k * P:(k + 1) * P]        # [NPC, P] lhsT
                ej = expj[:, k * NPC:(k + 1) * NPC]      # [P, NPC] lhsT

                # per-slot start/end expansion
                st_ps = spool.tile([P, 1], F32, tag="ps_small")
                en_ps = spool.tile([P, 1], F32, tag="ps_small")
                nc.tensor.matmul(out=st_ps[:], lhsT=ejt, rhs=st_f[:],
                                 start=True, stop=True)
                nc.tensor.matmul(out=en_ps[:], lhsT=ejt, rhs=en_f[:],
                                 start=True, stop=True)
                offs_f = wpool.tile([P, 1], F32, tag="offs_f")
                nc.vector.tensor_add(out=offs_f[:], in0=st_ps[:], in1=woff[:])
                valid = wpool.tile([P, 1], F32, tag="valid")
                nc.vector.tensor_tensor(out=valid[:], in0=offs_f[:],
                                        in1=en_ps[:], op=mybir.AluOpType.is_lt)
                offs_i = wpool.tile([P, 1], I32, tag="offs_i")
                nc.vector.tensor_copy(out=offs_i[:], in_=offs_f[:])

                # gather src ids, then x rows
                srcv = wpool.tile([P, 1], I32, tag="srcv")
                nc.gpsimd.indirect_dma_start(
                    out=srcv[:], out_offset=None, in_=srcs_d[:],
                    in_offset=IndirectOffsetOnAxis(ap=offs_i[:], axis=0))
                xsel = wpool.tile([P, D], F32, tag="xsel")
                nc.gpsimd.indirect_dma_start(
                    out=xsel[:], out_offset=None, in_=x_d[:],
                    in_offset=IndirectOffsetOnAxis(ap=srcv[:], axis=0))

                # x_sel^T (two 128x128 transposes)
                xt = wpool.tile([P, D], F32, tag="xt")
                for t in range(2):
                    xt_ps = spool.tile([P, P], F32, tag="ps_small")
                    nc.tensor.transpose(out=xt_ps[:],
                                        in_=xsel[:, t * P:(t + 1) * P],
                                        identity=ident[:])
                    nc.vector.tensor_copy(out=xt[:, t * P:(t + 1) * P],
                                          in_=xt_ps[:])

                # K/V projections of gathered rows
                k_ps = ppool.tile([P, D], F32, tag="ps_k")
                v_ps = ppool.tile([P, D], F32, tag="ps_v")
                for t in range(2):
                    nc.tensor.matmul(out=k_ps[:], lhsT=xt[:, t * P:(t + 1) * P],
                                     rhs=wk[:, t * D:(t + 1) * D],
                                     start=(t == 0), stop=(t == 1))
                for t in range(2):
                    nc.tensor.matmul(out=v_ps[:], lhsT=xt[:, t * P:(t + 1) * P],
                                     rhs=wv[:, t * D:(t + 1) * D],
                                     start=(t == 0), stop=(t == 1))
                ksel = wpool.tile([P, D], F32, tag="ksel")
                nc.vector.tensor_copy(out=ksel[:], in_=k_ps[:])
                vsel = wpool.tile([P, D], F32, tag="vsel")
                nc.vector.tensor_copy(out=vsel[:], in_=v_ps[:])

                # qe = q row per slot
                qe_ps = ppool.tile([P, D], F32, tag="ps_qe")
                nc.tensor.matmul(out=qe_ps[:], lhsT=ejt, rhs=qm[:],
                                 start=True, stop=True)

                # scores s[p,h], e = exp(s/8) * valid
                prod = wpool.tile([P, D], F32, tag="prod")
                nc.vector.tensor_mul(out=prod[:], in0=ksel[:], in1=qe_ps[:])
                s = wpool.tile([P, H], F32, tag="s")
                nc.vector.tensor_reduce(
                    out=s[:], in_=prod[:].rearrange("p (h d) -> p h d", h=H),
                    axis=mybir.AxisListType.X, op=mybir.AluOpType.add)
                e = wpool.tile([P, H], F32, tag="e")
                nc.scalar.activation(out=e[:], in_=s[:],
                                     func=mybir.ActivationFunctionType.Exp,
                                     scale=float(1.0 / np.sqrt(DK)))
                agg = wpool.tile([P, D + H + 1], F32, tag="agg")
                nc.vector.tensor_scalar_mul(agg[:, D:D + H], e[:], valid[:])
                nc.vector.tensor_copy(out=agg[:, D + H:D + H + 1], in_=valid[:])
                # w = v * alpha-weights (per head)
                for h in range(H):
                    nc.vector.tensor_scalar_mul(
                        agg[:, h * DK:(h + 1) * DK],
                        vsel[:, h * DK:(h + 1) * DK],
                        agg[:, D + h:D + h + 1])
                # per-node reduction (numer | denom | count)
                agg_ps = spool.tile([NPC, D + H + 1], F32, tag="ps_small")
                nc.tensor.matmul(out=agg_ps[:], lhsT=ej, rhs=agg[:],
                                 start=True, stop=True)
                if n_chunks == 1:
                    nc.vector.tensor_copy(out=acc[:], in_=agg_ps[:])
                elif k == 0:
                    nc.vector.tensor_copy(out=acc[:], in_=agg_ps[:])
                else:
                    nc.vector.tensor_add(out=acc[:], in0=acc[:], in1=agg_ps[:])

            # ---- normalize: out_node = numer / max(denom, empty-guard) ----
            iszero = wpool.tile([NPC, 1], F32, tag="iszero")
            nc.vector.tensor_scalar(out=iszero[:], in0=acc[:, D + H:D + H + 1],
                                    scalar1=0.5, scalar2=None,
                                    op0=mybir.AluOpType.is_lt)
            den = wpool.tile([NPC, H], F32, tag="den")
            nc.vector.tensor_scalar(out=den[:], in0=acc[:, D:D + H],
                                    scalar1=iszero[:], scalar2=None,
                                    op0=mybir.AluOpType.add)
            rec = wpool.tile([NPC, H], F32, tag="rec")
            nc.vector.reciprocal(out=rec[:], in_=den[:])
            onode = wpool.tile([NPC, D], F32, tag="onode")
            for h in range(H):
                nc.vector.tensor_scalar_mul(
                    onode[:, h * DK:(h + 1) * DK],
                    acc[:, h * DK:(h + 1) * DK], rec[:, h:h + 1])

            # ---- r = out_node @ WO.T ----
            ot = wpool.tile([P, 2 * NPC], F32, tag="ot")
            for t in range(2):
                ot_ps = spool.tile([P, NPC], F32, tag="ps_small")
                nc.tensor.transpose(out=ot_ps[:],
                                    in_=onode[:, t * P:(t + 1) * P],
                                    identity=ident[:NPC, :NPC])
                nc.vector.tensor_copy(out=ot[:, t * NPC:(t + 1) * NPC],
                                      in_=ot_ps[:])
            r_ps = spool.tile([NPC, D], F32, tag="ps_small")
            for t in range(2):
                nc.tensor.matmul(out=r_ps[:], lhsT=ot[:, t * NPC:(t + 1) * NPC],
                                 rhs=wo[:, t * D:(t + 1) * D],
                                 start=(t == 0), stop=(t == 1))
            r_sb = wpool.tile([NPC, D], F32, tag="r_sb")
            nc.vector.tensor_copy(out=r_sb[:], in_=r_ps[:])
            nc.sync.dma_start(out=out_d[:], in_=r_sb[:])

    nc.compile()
    return nc


def _run_general(query, x, sorted_src, row_ptr, glob, cap, WQ, WK, WV, WO):
    """General fallback: arbitrary glob_idx values / larger caps."""
    expjt, expj, woff, nch = _expanders(cap)
    srcs_pad = np.concatenate(
        [sorted_src, np.zeros(cap, np.int32)]).reshape(NE + cap, 1)
    rp2 = np.ascontiguousarray(row_ptr.reshape(NV + 1, 1))
    shared = dict(
        x=x, srcs=srcs_pad, row_ptr=rp2, query=query,
        wqt=np.ascontiguousarray(WQ.T), wkt=np.ascontiguousarray(WK.T),
        wvt=np.ascontiguousarray(WV.T), wot=np.ascontiguousarray(WO.T),
        expjt=expjt, expj=expj,
        win_off=np.ascontiguousarray(woff.reshape(P, 1)),
        ident=np.eye(P, dtype=np.float32))

    in_maps = []
    for c in range(NCORES):
        mine = glob[c::NCORES]
        mgs = mine.astype(np.int32).reshape(NPC, 1)
        mge = (mine + 1).astype(np.int32).reshape(NPC, 1)
        selc = np.zeros((B, NPC), np.float32)
        selc[c + NCORES * np.arange(NPC), np.arange(NPC)] = 1.0
        in_maps.append(dict(shared, my_glob_s=mgs, my_glob_e=mge, sel=selc))

    key = ("gen", cap)
    if key not in _cache:
        _cache[key] = _build_general(cap)
    nc = _cache[key]

    trace = bool(int(os.environ.get("BASSK_TRACE", "0")))
    return run_bass_kernel_spmd(nc, in_maps, core_ids=list(range(NCORES)),
                                trace=trace)


# revision 51
# speedup vs baseline: 1.1582x; 1.1582x over previous
"""Bass/Trainium2 kernel for nn_DecoderAttention (gnn message passing).

Math: q = query @ WQ.T is scattered to the 64 global nodes (glob_idx) and is
zero everywhere else, and the output only reads out[glob_idx].  Therefore only
edges whose dst is a global node contribute to the result.  Host-side we
partition the edge list by dst (CSR sort, as the sharding hint prescribes) and
shard the 64 global nodes across the 8 cores (node list i::8 -> core i); the
per-core input shard is the <=128 x rows referenced by that core's edges,
re-laid-out host-side into one contiguous bf16 header block (one direct DMA,
no on-device gather).  Each core projects its gathered rows with K/V, does the
per-node masked softmax and aggregation, and applies the output projection for
its 8 rows; all tensor FLOPs of the module run on device in bf16 (tolerance
2e-2; measured rel err ~1e-3).

A general fallback using indirect row_ptr/src/x gathers handles arbitrary
glob_idx / caps that overflow the fast layout.
"""

import os

import numpy as np
import ml_dtypes

import concourse.bacc as bacc
import concourse.mybir as mybir
from concourse.bass import IndirectOffsetOnAxis
from concourse.bass_utils import run_bass_kernel_spmd
from concourse.tile import TileContext

BF16 = ml_dtypes.bfloat16


class _SlimTailTileContext(TileContext):
    """TileContext whose kernel tail skips the final all-engine barrier.

    The standard tail is drain -> barrier -> sem clears -> barrier.  The last
    barrier only isolates the clears from code following the TileContext in
    multi-kernel modules; this NEFF ends right after, and each engine halts
    only once its own instruction stream (including the clears) completes, so
    it is dead weight here."""

    def _drain_and_barrier(self, tick_clock, wait_clock):
        from concourse.tile import ScopedClock

        nc = self.nc
        drain_inst = nc.sync.drain()
        wait_clock.add_sem_waits(
            drain_inst.ins, ScopedClock({None: tick_clock.global_clock})
        )
        # One drain->sem hop orders the gpsimd sem clears after all work,
        # instead of the full (expensive) all-engine EVSEM butterfly.
        done = nc.alloc_semaphore("tail_done")
        drain_inst.then_inc(done, 1)
        nc.gpsimd.wait_ge(done, 1)
        assert self.sems is not None
        popped = nc._tile_sem_poison_stack.pop()
        assert popped is self._sem_poison
        # sem_clear only (skip clear_and_free's dma_reset: each NEFF load
        # re-initializes the DMA rings, and the reset machinery is the
        # dominant cost of the kernel tail)
        from concourse.bass import compact_to_ranges
        nums = sorted(s.num if hasattr(s, "num") else s
                      for s in list(self.sems.allocated().values()) + [done])
        for r in compact_to_ranges(nums):
            nc.gpsimd.sem_clear(r)

D = 256
H = 4
DK = 64
NV = 40000
NE = 320000
B = 64
NCORES = 8
P = 128
NPC = B // NCORES  # nodes (output rows) per core: 8

F32 = mybir.dt.float32
I32 = mybir.dt.int32
BF = mybir.dt.bfloat16

_cache: dict = {}
_gc_min_zero = [True]  # does some global node have zero incoming edges?

last_results = None  # BassKernelResults of the most recent run (for harness)

# fast-path hdr column layout (all bf16)
C_XSEL = 0                      # [:, 0:256]   gathered x rows, pre-transposed
                                #   host-side: hdr[d, t*128+p] = x_sel[p, t*128+d]
C_Q = C_XSEL + D                # [:, 256:272] query^T chunks (2 x [128, 8])
C_EXPJ = C_Q + 2 * NPC          # [:, 272:280] slot->node one-hot (lhsT)
C_NEGB = C_EXPJ + NPC           # [:, 280]     exp bias: (valid-1)*30
C_VALID = C_NEGB + 1            # [:, 281]     slot validity 0/1
C_MLO = C_VALID + 1             # [:, 282]     partition < 64 mask
C_MHI = C_MLO + 1               # [:, 283]     partition >= 64 mask
HC = C_MHI + 1                  # 284

# wall column layout (all bf16, shared across cores); each weight is a
# separate DMA spread over the three DMA-capable rings so they land in
# the order the compute chain consumes them (wq, wv, wo).  WK ships
# separately in natural (out-dim major) head-packed layout for the
# score-side fold (see CT below).
W_WQ = 0                        # [:, 0:512]      WQ.T d-chunks
W_WV = W_WQ + 2 * D             # [:, 512:1024]   WV.T d-chunks
W_WO = W_WV + 2 * D             # [:, 1024:1536]  WO.T d-chunks
WC = W_WO + 2 * D               # 1536


def _build_fast(guard: bool):
    """Fast-path SPMD program: direct-DMA inputs only, bf16 compute.

    Requires glob_idx == arange(64) and each core's <=128 relevant edges
    pre-gathered host-side into hdr (see kernel()).  guard=True adds the
    empty-node denominator guard (only needed when some global node has
    no incoming edges)."""
    nc = bacc.Bacc("TRN2", target_bir_lowering=False, debug=False,
                   num_devices=NCORES)

    hdr_d = nc.dram_tensor("hdr", [P, HC], BF, kind="ExternalInput")
    wall_d = nc.dram_tensor("wall", [P, WC], BF, kind="ExternalInput")
    wkn_d = nc.dram_tensor("wkn", [P, 2 * D], BF, kind="ExternalInput")
    # output is r^T: out_r[d, t*8+j] = r[j, t*128+d]
    out_d = nc.dram_tensor("out_r", [P, 2 * NPC], F32, kind="ExternalOutput")

    NAGG = D + H + 1 if guard else D + H

    with _SlimTailTileContext(nc) as tc:
        with (
            tc.tile_pool(name="sbuf", bufs=1) as sb,
            tc.tile_pool(name="psum", bufs=1, space="PSUM") as pp,
            tc.tile_pool(name="psmall", bufs=2, space="PSUM") as ps,
        ):
            hdr = sb.tile([P, HC], BF, tag="hdr")
            nc.sync.dma_start(out=hdr[:], in_=hdr_d[:])
            wall = sb.tile([P, WC], BF, tag="wall")
            wkn = sb.tile([P, 2 * D], BF, tag="wkn")
            nc.scalar.dma_start(out=wall[:, W_WQ:W_WQ + 2 * D],
                                in_=wall_d[:, W_WQ:W_WQ + 2 * D])
            nc.scalar.dma_start(out=wkn[:], in_=wkn_d[:])
            nc.gpsimd.dma_start(out=wall[:, W_WV:W_WV + 2 * D],
                                in_=wall_d[:, W_WV:W_WV + 2 * D])
            nc.sync.dma_start(out=wall[:, W_WO:W_WO + 2 * D],
                              in_=wall_d[:, W_WO:W_WO + 2 * D])
            # identity built on-chip (gpsimd finishes before the DMA rings
            # even come up), keeping the DMA window for real payload
            ident = sb.tile([P, P], BF, tag="ident")
            from concourse.masks import make_identity
            make_identity(nc, ident[:])

            ej = hdr[:, C_EXPJ:C_EXPJ + NPC]

            # exp bias to f32 (activation bias operand)
            negb = sb.tile([P, 1], F32, tag="negb")
            nc.vector.tensor_copy(out=negb[:], in_=hdr[:, C_NEGB:C_NEGB + 1])

            # x_sel^T arrives pre-transposed in the hdr (host layout)
            xt = hdr[:, C_XSEL:C_XSEL + D]

            # qmT[hk, j] = (query_mine @ WQ.T)^T, computed directly in
            # transposed layout (hk on partitions) -- no PE transposes
            qmt_ps = ps.tile([P, 2 * NPC], F32, tag="ps_small")
            for u in range(2):
                for t in range(2):
                    nc.tensor.matmul(
                        out=qmt_ps[:, u * NPC:(u + 1) * NPC],
                        lhsT=wall[:, W_WQ + t * D + u * P:
                                  W_WQ + t * D + (u + 1) * P],
                        rhs=hdr[:, C_Q + t * NPC:C_Q + (t + 1) * NPC],
                        start=(t == 0), stop=(t == 1))
            # zero-pad qmT per head half so each 128-deep contraction chunk
            # only sees its own head's 64 rows: qmtp cols (u, hl, j).
            # Masked straight out of PSUM, split across vector and gpsimd.
            hmask = sb.tile([P, 2], F32, tag="hmask")
            nc.vector.tensor_copy(out=hmask[:], in_=hdr[:, C_MLO:C_MHI + 1])
            qmtp = sb.tile([P, 4 * NPC], BF, tag="qmtp")
            for u in range(2):
                for hl in range(2):
                    nc.vector.tensor_scalar(
                        out=qmtp[:, (u * 2 + hl) * NPC:
                                 (u * 2 + hl + 1) * NPC],
                        in0=qmt_ps[:, u * NPC:(u + 1) * NPC],
                        scalar1=hmask[:, hl:hl + 1], scalar2=None,
                        op0=mybir.AluOpType.mult)

            # CT[d, (h,j)] = sum_k WK[h*64+k, d] * qm[j, h*64+k]: the k-proj
            # folded into the query side, so scores are one small matmul on
            # the gathered rows instead of a full K projection.  wkn holds
            # WK natural 128-row chunks; head separation comes from qmtp.
            ct_ps = pp.tile([P, 2 * H * NPC], F32, tag="ps_ct")
            for t in range(2):
                for u in range(2):
                    nc.tensor.matmul(
                        out=ct_ps[:, t * H * NPC + u * 2 * NPC:
                                  t * H * NPC + (u + 1) * 2 * NPC],
                        lhsT=wkn[:, u * D + t * P:u * D + (t + 1) * P],
                        rhs=qmtp[:, u * 2 * NPC:(u + 1) * 2 * NPC],
                        start=True, stop=True)
            ct = sb.tile([P, 2 * H * NPC], BF, tag="ct")
            nc.vector.tensor_copy(out=ct[:], in_=ct_ps[:])

            # per-slot scores for all (head, node) pairs, then select own node
            s_ps = ps.tile([P, H * NPC], F32, tag="ps_small")
            for t in range(2):
                nc.tensor.matmul(out=s_ps[:], lhsT=xt[:, t * P:(t + 1) * P],
                                 rhs=ct[:, t * H * NPC:(t + 1) * H * NPC],
                                 start=(t == 0), stop=(t == 1))
            sm = sb.tile([P, H * NPC], F32, tag="sm")
            nc.vector.tensor_tensor(
                out=sm[:].rearrange("p (h j) -> p h j", h=H),
                in0=s_ps[:].rearrange("p (h j) -> p h j", h=H),
                in1=ej.rearrange("p (o j) -> p o j", o=1)
                    .to_broadcast([P, H, NPC]),
                op=mybir.AluOpType.mult)
            s = sb.tile([P, H], F32, tag="s")
            nc.vector.tensor_reduce(
                out=s[:], in_=sm[:].rearrange("p (h j) -> p h j", h=H),
                axis=mybir.AxisListType.X, op=mybir.AluOpType.add)

            # V projection of the gathered rows
            v_ps = pp.tile([P, D], F32, tag="ps_v")
            for t in range(2):
                nc.tensor.matmul(out=v_ps[:], lhsT=xt[:, t * P:(t + 1) * P],
                                 rhs=wall[:, W_WV + t * D:W_WV + (t + 1) * D],
                                 start=(t == 0), stop=(t == 1))
            # agg = [e-weighted v | e (| valid)]  (bf16 so the reduction
            # matmul runs at full PE rate; accumulation is f32 in PSUM)
            agg = sb.tile([P, NAGG], BF, tag="agg")
            nc.scalar.activation(out=agg[:, D:D + H], in_=s[:],
                                 func=mybir.ActivationFunctionType.Exp,
                                 bias=negb[:],
                                 scale=float(1.0 / np.sqrt(DK)))
            if guard:
                nc.vector.tensor_copy(out=agg[:, D + H:D + H + 1],
                                      in_=hdr[:, C_VALID:C_VALID + 1])
            nc.vector.tensor_tensor(
                out=agg[:, 0:D].rearrange("p (h d) -> p h d", h=H),
                in0=v_ps[:].rearrange("p (h d) -> p h d", h=H),
                in1=agg[:, D:D + H].to_broadcast([P, H, DK]),
                op=mybir.AluOpType.mult)

            # per-node reduction: [numer | denom (| count)]
            acc_ps = ps.tile([NPC, NAGG], F32, tag="ps_small")
            nc.tensor.matmul(out=acc_ps[:], lhsT=ej, rhs=agg[:],
                             start=True, stop=True)

            rec = sb.tile([NPC, H], F32, tag="rec")
            if guard:
                # guard empty nodes: denom += (count == 0)
                iszero = sb.tile([NPC, 1], F32, tag="iszero")
                nc.vector.tensor_scalar(out=iszero[:],
                                        in0=acc_ps[:, D + H:D + H + 1],
                                        scalar1=0.5, scalar2=None,
                                        op0=mybir.AluOpType.is_lt)
                den = sb.tile([NPC, H], F32, tag="den")
                nc.vector.tensor_scalar(out=den[:], in0=acc_ps[:, D:D + H],
                                        scalar1=iszero[:], scalar2=None,
                                        op0=mybir.AluOpType.add)
                nc.vector.reciprocal(out=rec[:], in_=den[:])
            else:
                nc.vector.reciprocal(out=rec[:], in_=acc_ps[:, D:D + H])
            onode = sb.tile([NPC, D], BF, tag="onode")
            nc.vector.tensor_tensor(
                out=onode[:].rearrange("p (h d) -> p h d", h=H),
                in0=acc_ps[:, 0:D].rearrange("p (h d) -> p h d", h=H),
                in1=rec[:].to_broadcast([NPC, H, DK]),
                op=mybir.AluOpType.mult)

            # r = out_node @ WO.T
            ot_ps = ps.tile([P, 2 * NPC], BF, tag="ps_small")
            for t in range(2):
                nc.tensor.transpose(out=ot_ps[:, t * NPC:(t + 1) * NPC],
                                    in_=onode[:, t * P:(t + 1) * P],
                                    identity=ident[0:NPC, 0:NPC])
            ot = sb.tile([P, 2 * NPC], BF, tag="ot")
            nc.vector.tensor_copy(out=ot[:], in_=ot_ps[:])
            # r^T directly (full-partition copies and a tiny out DMA):
            # rT[d, (t,j)] = r[j, t*128+d] = sum_u WO[t*128+d, u*128+d'] ...
            r_ps = ps.tile([P, 2 * NPC], F32, tag="ps_small")
            for t in range(2):
                for u in range(2):
                    nc.tensor.matmul(
                        out=r_ps[:, t * NPC:(t + 1) * NPC],
                        lhsT=wall[:, W_WO + u * D + t * P:
                                  W_WO + u * D + (t + 1) * P],
                        rhs=ot[:, u * NPC:(u + 1) * NPC],
                        start=(u == 0), stop=(u == 1))
            r_sb = sb.tile([P, 2 * NPC], F32, tag="r_sb")
            nc.vector.tensor_copy(out=r_sb[:], in_=r_ps[:])
            nc.sync.dma_start(out=out_d[:], in_=r_sb[:])

    nc.compile()
    return nc


def kernel(query, x, WQ, WK, WV, WO, src, dst, glob_idx):
    global last_results
    query = np.ascontiguousarray(np.asarray(query, dtype=np.float32))
    x = np.ascontiguousarray(np.asarray(x, dtype=np.float32))
    src32 = np.asarray(src, dtype=np.int32)
    dst32 = np.asarray(dst, dtype=np.int32)
    glob = np.asarray(glob_idx, dtype=np.int32)
    WQ = np.asarray(WQ, np.float32)
    WK = np.asarray(WK, np.float32)
    WV = np.asarray(WV, np.float32)
    WO = np.asarray(WO, np.float32)

    # partition (CSR-sort) edge list by dst shard (dst % 8), then dst
    shard = dst32 % NCORES
    order = np.lexsort((dst32, shard))
    s_src = src32[order]
    s_dst = dst32[order]
    s_shard = shard[order]
    shard_start = np.searchsorted(s_shard, np.arange(NCORES + 1))

    # per-global-node edge counts (for capacity + fast-path check)
    rel = dst32 < B
    gc = np.bincount(dst32[rel], minlength=B) if rel.any() else \
        np.zeros(B, np.int64)

    cap16_ok = gc.max() <= 16 if len(gc) else True
    pref_ok = all(gc[c::NCORES].sum() <= P for c in range(NCORES))
    _gc_min_zero[0] = bool(gc.min() == 0) if len(gc) else True
    fast = (np.array_equal(glob, np.arange(B, dtype=glob.dtype))
            and cap16_ok and pref_ok
            and not bool(int(os.environ.get("BASSK_FORCE_GENERAL", "0"))))

    if fast:
        res = _run_fast(query, x, s_src, s_dst, shard_start, WQ, WK, WV, WO)
    else:
        perm = np.argsort(dst32, kind="stable")
        sorted_src = np.ascontiguousarray(src32[perm])
        sorted_dst = dst32[perm]
        row_ptr = np.searchsorted(sorted_dst,
                                  np.arange(NV + 1)).astype(np.int32)
        gcnt = int((row_ptr[glob + 1] - row_ptr[glob]).max()) if len(glob) \
            else 0
        cap = 16
        while cap < gcnt:
            cap *= 2
        res = _run_general(query, x, sorted_src, row_ptr, glob, cap,
                           WQ, WK, WV, WO)
    last_results = res
    if fast:
        # per-core out is r^T [128, (t, j)]: r_c[j, t*128+d] = out[d, t*8+j]
        outs = [np.transpose(
            np.asarray(res.results[c]["out_r"]).reshape(P, 2, NPC),
            (2, 1, 0)).reshape(NPC, D) for c in range(NCORES)]
    else:
        outs = [res.results[c]["out_r"] for c in range(NCORES)]
    return np.ascontiguousarray(
        np.stack(outs, axis=1).reshape(B, D).astype(np.float32))


def _run_fast(query, x, s_src, s_dst, shard_start, WQ, WK, WV, WO):
    cap = 16
    guard = bool(_gc_min_zero[0])

    # weight wall (shared): W^T d-chunks, bf16
    wall = np.zeros((P, WC), np.float32)
    for t in range(2):
        dd = slice(t * P, (t + 1) * P)
        wall[:, W_WQ + t * D:W_WQ + (t + 1) * D] = WQ.T[dd]
        wall[:, W_WV + t * D:W_WV + (t + 1) * D] = WV.T[dd]
        wall[:, W_WO + t * D:W_WO + (t + 1) * D] = WO.T[dd]
    wall_bf = np.ascontiguousarray(wall.astype(BF16))
    # WK natural 128-row chunks side by side: wkn[p, u*D + d] = WK[u*128+p, d]
    wkn = np.ascontiguousarray(
        np.concatenate([WK[0:P, :], WK[P:2 * P, :]], axis=1).astype(BF16))

    nos = np.arange(P) // cap
    expj = np.zeros((P, NPC), np.float32)
    expj[np.arange(P), nos] = 1.0

    qT = query.T  # (D, B)
    in_maps = []
    for c in range(NCORES):
        lo, hi = int(shard_start[c]), int(shard_start[c + 1])
        sh_dst = s_dst[lo:hi]
        sh_src = s_src[lo:hi]
        n = hi - lo
        # shard-local row_ptr over my 8 nodes (c, c+8, .., c+56) + end
        my_nodes = c + NCORES * np.arange(NPC + 1)  # node c+64 bounds the end
        rp9 = np.searchsorted(sh_dst, my_nodes).astype(np.int64)
        offs_col = rp9[nos] + np.arange(P) % cap
        valid_col = (offs_col < rp9[nos + 1]).astype(np.float32)
        if n > 0:
            slot_src = np.where(offs_col < n,
                                sh_src[np.minimum(offs_col, n - 1)], 0)
        else:
            slot_src = np.zeros(P, np.int64)
        hdr = np.zeros((P, HC), np.float32)
        xs = x[slot_src]  # [128 slots, 256]; ship transposed per d-chunk
        for t in range(2):
            hdr[:, C_XSEL + t * P:C_XSEL + (t + 1) * P] = \
                xs[:, t * P:(t + 1) * P].T
        for t in range(2):
            hdr[:, C_Q + t * NPC:C_Q + (t + 1) * NPC] = \
                qT[t * P:(t + 1) * P, c::NCORES]
        hdr[:, C_EXPJ:C_EXPJ + NPC] = expj
        hdr[:, C_NEGB] = (valid_col - 1.0) * 30.0
        hdr[:, C_VALID] = valid_col
        hdr[:, C_MLO] = (np.arange(P) < DK).astype(np.float32)
        hdr[:, C_MHI] = (np.arange(P) >= DK).astype(np.float32)
        in_maps.append(dict(wall=wall_bf, wkn=wkn,
                            hdr=np.ascontiguousarray(hdr.astype(BF16))))

    key = ("fastbf", guard)
    if key not in _cache:
        _cache[key] = _build_fast(guard)
    nc = _cache[key]

    trace = bool(int(os.environ.get("BASSK_TRACE", "0")))
    return run_bass_kernel_spmd(nc, in_maps, core_ids=list(range(NCORES)),
                                trace=trace)


# ---------------------------------------------------------------------------
# general fallback (from validated v1 program)
# ---------------------------------------------------------------------------

def _expanders(cap):
    nslots = NPC * cap
    nch = nslots // P
    npc_chunk = P // cap
    expjt = np.zeros((NPC, P * nch), np.float32)
    expj = np.zeros((P, NPC * nch), np.float32)
    for k in range(nch):
        j_of_p = np.arange(P) // cap + k * npc_chunk
        expjt[j_of_p, k * P + np.arange(P)] = 1.0
        expj[np.arange(P), k * NPC + j_of_p] = 1.0
    woff = (np.arange(P) % cap).astype(np.float32)
    return expjt, expj, woff, nch


def _build_general(cap: int):
    """Build the SPMD Bass program. cap = edge slots per node (power of two,
    NPC*cap multiple of 128)."""
    nslots = NPC * cap
    n_chunks = nslots // P
    assert nslots % P == 0
    npc_chunk = P // cap  # nodes per 128-slot chunk

    nc = bacc.Bacc("TRN2", target_bir_lowering=False, debug=False,
                   num_devices=NCORES)

    # ---- DRAM I/O ----
    x_d = nc.dram_tensor("x", [NV, D], F32, kind="ExternalInput")
    srcs_d = nc.dram_tensor("srcs", [NE + cap, 1], I32, kind="ExternalInput")
    rp_d = nc.dram_tensor("row_ptr", [NV + 1, 1], I32, kind="ExternalInput")
    qy_d = nc.dram_tensor("query", [B, D], F32, kind="ExternalInput")
    wqt_d = nc.dram_tensor("wqt", [D, D], F32, kind="ExternalInput")
    wkt_d = nc.dram_tensor("wkt", [D, D], F32, kind="ExternalInput")
    wvt_d = nc.dram_tensor("wvt", [D, D], F32, kind="ExternalInput")
    wot_d = nc.dram_tensor("wot", [D, D], F32, kind="ExternalInput")
    sel_d = nc.dram_tensor("sel", [B, NPC], F32, kind="ExternalInput")
    expjt_d = nc.dram_tensor("expjt", [NPC, P * n_chunks], F32,
                             kind="ExternalInput")
    expj_d = nc.dram_tensor("expj", [P, NPC * n_chunks], F32,
                            kind="ExternalInput")
    woff_d = nc.dram_tensor("win_off", [P, 1], F32, kind="ExternalInput")
    ident_d = nc.dram_tensor("ident", [P, P], F32, kind="ExternalInput")
    mgs_d = nc.dram_tensor("my_glob_s", [NPC, 1], I32, kind="ExternalInput")
    mge_d = nc.dram_tensor("my_glob_e", [NPC, 1], I32, kind="ExternalInput")
    out_d = nc.dram_tensor("out_r", [NPC, D], F32, kind="ExternalOutput")

    with _SlimTailTileContext(nc) as tc:
        with (
            tc.tile_pool(name="const", bufs=1) as cpool,
            tc.tile_pool(name="work", bufs=1) as wpool,
            tc.tile_pool(name="psum", bufs=1, space="PSUM") as ppool,
            tc.tile_pool(name="psum_small", bufs=2, space="PSUM") as spool,
        ):
            # ---- constant / weight loads (issued early, overlap the chain) --
            qy = cpool.tile([B, D], F32, tag="qy")
            nc.sync.dma_start(out=qy[:], in_=qy_d[:])
            wq = cpool.tile([P, 2 * D], F32, tag="wq")  # [d-chunk t] at cols t*D
            wk = cpool.tile([P, 2 * D], F32, tag="wk")
            wv = cpool.tile([P, 2 * D], F32, tag="wv")
            wo = cpool.tile([P, 2 * D], F32, tag="wo")
            for t in range(2):
                nc.sync.dma_start(out=wq[:, t * D:(t + 1) * D],
                                  in_=wqt_d[t * P:(t + 1) * P, :])
                nc.sync.dma_start(out=wk[:, t * D:(t + 1) * D],
                                  in_=wkt_d[t * P:(t + 1) * P, :])
                nc.sync.dma_start(out=wv[:, t * D:(t + 1) * D],
                                  in_=wvt_d[t * P:(t + 1) * P, :])
                nc.sync.dma_start(out=wo[:, t * D:(t + 1) * D],
                                  in_=wot_d[t * P:(t + 1) * P, :])
            sel = cpool.tile([B, NPC], F32, tag="sel")
            nc.sync.dma_start(out=sel[:], in_=sel_d[:])
            expjt = cpool.tile([NPC, P * n_chunks], F32, tag="expjt")
            nc.sync.dma_start(out=expjt[:], in_=expjt_d[:])
            expj = cpool.tile([P, NPC * n_chunks], F32, tag="expj")
            nc.sync.dma_start(out=expj[:], in_=expj_d[:])
            woff = cpool.tile([P, 1], F32, tag="woff")
            nc.sync.dma_start(out=woff[:], in_=woff_d[:])
            ident = cpool.tile([P, P], F32, tag="ident")
            nc.sync.dma_start(out=ident[:], in_=ident_d[:])
            mgs = cpool.tile([NPC, 1], I32, tag="mgs")
            nc.sync.dma_start(out=mgs[:], in_=mgs_d[:])
            mge = cpool.tile([NPC, 1], I32, tag="mge")
            nc.sync.dma_start(out=mge[:], in_=mge_d[:])

            # ---- row_ptr[glob] and row_ptr[glob+1] (one indirect gather) ----
            st_i = wpool.tile([NPC, 1], I32, tag="st_i")
            nc.gpsimd.indirect_dma_start(
                out=st_i[:], out_offset=None, in_=rp_d[:],
                in_offset=IndirectOffsetOnAxis(ap=mgs[:], axis=0))
            en_i = wpool.tile([NPC, 1], I32, tag="en_i")
            nc.gpsimd.indirect_dma_start(
                out=en_i[:], out_offset=None, in_=rp_d[:],
                in_offset=IndirectOffsetOnAxis(ap=mge[:], axis=0))
            st_f = wpool.tile([NPC, 1], F32, tag="st_f")
            nc.vector.tensor_copy(out=st_f[:], in_=st_i[:])
            en_f = wpool.tile([NPC, 1], F32, tag="en_f")
            nc.vector.tensor_copy(out=en_f[:], in_=en_i[:])

            # ---- q_glob = query @ WQ.T ; q_mine = my 8 rows ----
            qyt = wpool.tile([P, 2 * B], F32, tag="qyt")  # query^T d-chunks
            for t in range(2):
                pt = spool.tile([P, B], F32, tag="ps_small")
                nc.tensor.transpose(out=pt[:], in_=qy[:, t * P:(t + 1) * P],
                                    identity=ident[:B, :B])
                nc.vector.tensor_copy(out=qyt[:, t * B:(t + 1) * B], in_=pt[:])
            qg_ps = ppool.tile([B, D], F32, tag="ps_qg")
            for t in range(2):
                nc.tensor.matmul(out=qg_ps[:], lhsT=qyt[:, t * B:(t + 1) * B],
                                 rhs=wq[:, t * D:(t + 1) * D],
                                 start=(t == 0), stop=(t == 1))
            qg = wpool.tile([B, D], F32, tag="qg")
            nc.vector.tensor_copy(out=qg[:], in_=qg_ps[:])
            qm_ps = spool.tile([NPC, D], F32, tag="ps_small")
            nc.tensor.matmul(out=qm_ps[:], lhsT=sel[:], rhs=qg[:],
                             start=True, stop=True)
            qm = wpool.tile([NPC, D], F32, tag="qm")
            nc.vector.tensor_copy(out=qm[:], in_=qm_ps[:])

            # ---- accumulator over chunks (numer | denom | count) ----
            acc = wpool.tile([NPC, D + H + 1], F32, tag="acc")

            for k in range(n_chunks):
                ejt = expjt[:, k * P:(k + 1) * P]        # [NPC, P] lhsT
                ej = expj[:, k * NPC:(k + 1) * NPC]      # [P, NPC] lhsT

                # per-slot start/end expansion
                st_ps = spool.tile([P, 1], F32, tag="ps_small")
                en_ps = spool.tile([P, 1], F32, tag="ps_small")
                nc.tensor.matmul(out=st_ps[:], lhsT=ejt, rhs=st_f[:],
                                 start=True, stop=True)
                nc.tensor.matmul(out=en_ps[:], lhsT=ejt, rhs=en_f[:],
                                 start=True, stop=True)
                offs_f = wpool.tile([P, 1], F32, tag="offs_f")
                nc.vector.tensor_add(out=offs_f[:], in0=st_ps[:], in1=woff[:])
                valid = wpool.tile([P, 1], F32, tag="valid")
                nc.vector.tensor_tensor(out=valid[:], in0=offs_f[:],
                                        in1=en_ps[:], op=mybir.AluOpType.is_lt)
                offs_i = wpool.tile([P, 1], I32, tag="offs_i")
                nc.vector.tensor_copy(out=offs_i[:], in_=offs_f[:])

                # gather src ids, then x rows
                srcv = wpool.tile([P, 1], I32, tag="srcv")
                nc.gpsimd.indirect_dma_start(
                    out=srcv[:], out_offset=None, in_=srcs_d[:],
                    in_offset=IndirectOffsetOnAxis(ap=offs_i[:], axis=0))
                xsel = wpool.tile([P, D], F32, tag="xsel")
                nc.gpsimd.indirect_dma_start(
                    out=xsel[:], out_offset=None, in_=x_d[:],
                    in_offset=IndirectOffsetOnAxis(ap=srcv[:], axis=0))

                # x_sel^T (two 128x128 transposes)
                xt = wpool.tile([P, D], F32, tag="xt")
                for t in range(2):
                    xt_ps = spool.tile([P, P], F32, tag="ps_small")
                    nc.tensor.transpose(out=xt_ps[:],
                                        in_=xsel[:, t * P:(t + 1) * P],
                                        identity=ident[:])
                    nc.vector.tensor_copy(out=xt[:, t * P:(t + 1) * P],
                                          in_=xt_ps[:])

                # K/V projections of gathered rows
                k_ps = ppool.tile([P, D], F32, tag="ps_k")
                v_ps = ppool.tile([P, D], F32, tag="ps_v")
                for t in range(2):
                    nc.tensor.matmul(out=k_ps[:], lhsT=xt[:, t * P:(t + 1) * P],
                                     rhs=wk[:, t * D:(t + 1) * D],
                                     start=(t == 0), stop=(t == 1))
                for t in range(2):
                    nc.tensor.matmul(out=v_ps[:], lhsT=xt[:, t * P:(t + 1) * P],
                                     rhs=wv[:, t * D:(t + 1) * D],
                                     start=(t == 0), stop=(t == 1))
                ksel = wpool.tile([P, D], F32, tag="ksel")
                nc.vector.tensor_copy(out=ksel[:], in_=k_ps[:])
                vsel = wpool.tile([P, D], F32, tag="vsel")
                nc.vector.tensor_copy(out=vsel[:], in_=v_ps[:])

                # qe = q row per slot
                qe_ps = ppool.tile([P, D], F32, tag="ps_qe")
                nc.tensor.matmul(out=qe_ps[:], lhsT=ejt, rhs=qm[:],
                                 start=True, stop=True)

                # scores s[p,h], e = exp(s/8) * valid
                prod = wpool.tile([P, D], F32, tag="prod")
                nc.vector.tensor_mul(out=prod[:], in0=ksel[:], in1=qe_ps[:])
                s = wpool.tile([P, H], F32, tag="s")
                nc.vector.tensor_reduce(
                    out=s[:], in_=prod[:].rearrange("p (h d) -> p h d", h=H),
                    axis=mybir.AxisListType.X, op=mybir.AluOpType.add)
                e = wpool.tile([P, H], F32, tag="e")
                nc.scalar.activation(out=e[:], in_=s[:],
                                     func=mybir.ActivationFunctionType.Exp,
                                     scale=float(1.0 / np.sqrt(DK)))
                agg = wpool.tile([P, D + H + 1], F32, tag="agg")
                nc.vector.tensor_scalar_mul(agg[:, D:D + H], e[:], valid[:])
                nc.vector.tensor_copy(out=agg[:, D + H:D + H + 1], in_=valid[:])
                # w = v * alpha-weights (per head)
                for h in range(H):
                    nc.vector.tensor_scalar_mul(
                        agg[:, h * DK:(h + 1) * DK],
                        vsel[:, h * DK:(h + 1) * DK],
                        agg[:, D + h:D + h + 1])
                # per-node reduction (numer | denom | count)
                agg_ps = spool.tile([NPC, D + H + 1], F32, tag="ps_small")
                nc.tensor.matmul(out=agg_ps[:], lhsT=ej, rhs=agg[:],
                                 start=True, stop=True)
                if n_chunks == 1:
                    nc.vector.tensor_copy(out=acc[:], in_=agg_ps[:])
                elif k == 0:
                    nc.vector.tensor_copy(out=acc[:], in_=agg_ps[:])
                else:
                    nc.vector.tensor_add(out=acc[:], in0=acc[:], in1=agg_ps[:])

            # ---- normalize: out_node = numer / max(denom, empty-guard) ----
            iszero = wpool.tile([NPC, 1], F32, tag="iszero")
            nc.vector.tensor_scalar(out=iszero[:], in0=acc[:, D + H:D + H + 1],
                                    scalar1=0.5, scalar2=None,
                                    op0=mybir.AluOpType.is_lt)
            den = wpool.tile([NPC, H], F32, tag="den")
            nc.vector.tensor_scalar(out=den[:], in0=acc[:, D:D + H],
                                    scalar1=iszero[:], scalar2=None,
                                    op0=mybir.AluOpType.add)
            rec = wpool.tile([NPC, H], F32, tag="rec")
            nc.vector.reciprocal(out=rec[:], in_=den[:])
            onode = wpool.tile([NPC, D], F32, tag="onode")
            for h in range(H):
                nc.vector.tensor_scalar_mul(
                    onode[:, h * DK:(h + 1) * DK],
                    acc[:, h * DK:(h + 1) * DK], rec[:, h:h + 1])

            # ---- r = out_node @ WO.T ----
            ot = wpool.tile([P, 2 * NPC], F32, tag="ot")
            for t in range(2):
                ot_ps = spool.tile([P, NPC], F32, tag="ps_small")
                nc.tensor.transpose(out=ot_ps[:],
                                    in_=onode[:, t * P:(t + 1) * P],
                                    identity=ident[:NPC, :NPC])
                nc.vector.tensor_copy(out=ot[:, t * NPC:(t + 1) * NPC],
                                      in_=ot_ps[:])
            r_ps = spool.tile([NPC, D], F32, tag="ps_small")
            for t in range(2):
                nc.tensor.matmul(out=r_ps[:], lhsT=ot[:, t * NPC:(t + 1) * NPC],
                                 rhs=wo[:, t * D:(t + 1) * D],
                                 start=(t == 0), stop=(t == 1))
            r_sb = wpool.tile([NPC, D], F32, tag="r_sb")
            nc.vector.tensor_copy(out=r_sb[:], in_=r_ps[:])
            nc.sync.dma_start(out=out_d[:], in_=r_sb[:])

    nc.compile()
    return nc


def _run_general(query, x, sorted_src, row_ptr, glob, cap, WQ, WK, WV, WO):
    """General fallback: arbitrary glob_idx values / larger caps."""
    expjt, expj, woff, nch = _expanders(cap)
    srcs_pad = np.concatenate(
        [sorted_src, np.zeros(cap, np.int32)]).reshape(NE + cap, 1)
    rp2 = np.ascontiguousarray(row_ptr.reshape(NV + 1, 1))
    shared = dict(
        x=x, srcs=srcs_pad, row_ptr=rp2, query=query,
        wqt=np.ascontiguousarray(WQ.T), wkt=np.ascontiguousarray(WK.T),
        wvt=np.ascontiguousarray(WV.T), wot=np.ascontiguousarray(WO.T),
        expjt=expjt, expj=expj,
        win_off=np.ascontiguousarray(woff.reshape(P, 1)),
        ident=np.eye(P, dtype=np.float32))

    in_maps = []
    for c in range(NCORES):
        mine = glob[c::NCORES]
        mgs = mine.astype(np.int32).reshape(NPC, 1)
        mge = (mine + 1).astype(np.int32).reshape(NPC, 1)
        selc = np.zeros((B, NPC), np.float32)
        selc[c + NCORES * np.arange(NPC), np.arange(NPC)] = 1.0
        in_maps.append(dict(shared, my_glob_s=mgs, my_glob_e=mge, sel=selc))

    key = ("gen", cap)
    if key not in _cache:
        _cache[key] = _build_general(cap)
    nc = _cache[key]

    trace = bool(int(os.environ.get("BASSK_TRACE", "0")))
    return run_bass_kernel_spmd(nc, in_maps, core_ids=list(range(NCORES)),
                                trace=trace)


# revision 54
# speedup vs baseline: 1.1592x; 1.0008x over previous
"""Bass/Trainium2 kernel for nn_DecoderAttention (gnn message passing).

Math: q = query @ WQ.T is scattered to the 64 global nodes (glob_idx) and is
zero everywhere else, and the output only reads out[glob_idx].  Therefore only
edges whose dst is a global node contribute to the result.  Host-side we
partition the edge list by dst (CSR sort, as the sharding hint prescribes) and
shard the 64 global nodes across the 8 cores (node list i::8 -> core i); the
per-core input shard is the <=128 x rows referenced by that core's edges,
re-laid-out host-side into one contiguous bf16 header block (one direct DMA,
no on-device gather).  Each core projects its gathered rows with K/V, does the
per-node masked softmax and aggregation, and applies the output projection for
its 8 rows; all tensor FLOPs of the module run on device in bf16 (tolerance
2e-2; measured rel err ~1e-3).

A general fallback using indirect row_ptr/src/x gathers handles arbitrary
glob_idx / caps that overflow the fast layout.
"""

import os

import numpy as np
import ml_dtypes

import concourse.bacc as bacc
import concourse.mybir as mybir
from concourse.bass import IndirectOffsetOnAxis
from concourse.bass_utils import run_bass_kernel_spmd
from concourse.tile import TileContext

BF16 = ml_dtypes.bfloat16


class _SlimTailTileContext(TileContext):
    """TileContext whose kernel tail skips the final all-engine barrier.

    The standard tail is drain -> barrier -> sem clears -> barrier.  The last
    barrier only isolates the clears from code following the TileContext in
    multi-kernel modules; this NEFF ends right after, and each engine halts
    only once its own instruction stream (including the clears) completes, so
    it is dead weight here."""

    def _drain_and_barrier(self, tick_clock, wait_clock):
        from concourse.tile import ScopedClock

        nc = self.nc
        drain_inst = nc.sync.drain()
        wait_clock.add_sem_waits(
            drain_inst.ins, ScopedClock({None: tick_clock.global_clock})
        )
        # One drain->sem hop orders the gpsimd sem clears after all work,
        # instead of the full (expensive) all-engine EVSEM butterfly.
        done = nc.alloc_semaphore("tail_done")
        drain_inst.then_inc(done, 1)
        nc.gpsimd.wait_ge(done, 1)
        assert self.sems is not None
        popped = nc._tile_sem_poison_stack.pop()
        assert popped is self._sem_poison
        # sem_clear only (skip clear_and_free's dma_reset: each NEFF load
        # re-initializes the DMA rings, and the reset machinery is the
        # dominant cost of the kernel tail)
        from concourse.bass import compact_to_ranges
        nums = sorted(s.num if hasattr(s, "num") else s
                      for s in list(self.sems.allocated().values()) + [done])
        for r in compact_to_ranges(nums):
            nc.gpsimd.sem_clear(r)

D = 256
H = 4
DK = 64
NV = 40000
NE = 320000
B = 64
NCORES = 8
P = 128
NPC = B // NCORES  # nodes (output rows) per core: 8

F32 = mybir.dt.float32
I32 = mybir.dt.int32
BF = mybir.dt.bfloat16

_cache: dict = {}
_gc_min_zero = [True]  # does some global node have zero incoming edges?

last_results = None  # BassKernelResults of the most recent run (for harness)

# fast-path hdr column layout (all bf16)
C_XSEL = 0                      # [:, 0:256]   gathered x rows, pre-transposed
                                #   host-side: hdr[d, t*128+p] = x_sel[p, t*128+d]
C_Q = C_XSEL + D                # [:, 256:272] query^T chunks (2 x [128, 8])
C_EXPJ = C_Q + 2 * NPC          # [:, 272:280] slot->node one-hot (lhsT)
C_NEGB = C_EXPJ + NPC           # [:, 280]     exp bias: (valid-1)*30
C_VALID = C_NEGB + 1            # [:, 281]     slot validity 0/1
C_MLO = C_VALID + 1             # [:, 282]     partition < 64 mask
C_MHI = C_MLO + 1               # [:, 283]     partition >= 64 mask
HC = C_MHI + 1                  # 284

# wall column layout (all bf16, shared across cores); each weight is a
# separate DMA spread over the three DMA-capable rings so they land in
# the order the compute chain consumes them (wq, wv, wo).  WK ships
# separately in natural (out-dim major) head-packed layout for the
# score-side fold (see CT below).
W_WQ = 0                        # [:, 0:512]      WQ.T d-chunks
W_WV = W_WQ + 2 * D             # [:, 512:1024]   WV.T d-chunks
W_WO = W_WV + 2 * D             # [:, 1024:1536]  WO.T d-chunks
WC = W_WO + 2 * D               # 1536


def _build_fast(guard: bool):
    """Fast-path SPMD program: direct-DMA inputs only, bf16 compute.

    Requires glob_idx == arange(64) and each core's <=128 relevant edges
    pre-gathered host-side into hdr (see kernel()).  guard=True adds the
    empty-node denominator guard (only needed when some global node has
    no incoming edges)."""
    nc = bacc.Bacc("TRN2", target_bir_lowering=False, debug=False,
                   num_devices=NCORES)

    hdr_d = nc.dram_tensor("hdr", [P, HC], BF, kind="ExternalInput")
    wall_d = nc.dram_tensor("wall", [P, WC], BF, kind="ExternalInput")
    wkn_d = nc.dram_tensor("wkn", [P, 2 * D], BF, kind="ExternalInput")
    # output is r^T: out_r[d, t*8+j] = r[j, t*128+d]
    out_d = nc.dram_tensor("out_r", [P, 2 * NPC], F32, kind="ExternalOutput")

    NAGG = D + H + 1 if guard else D + H

    with _SlimTailTileContext(nc) as tc:
        with (
            tc.tile_pool(name="sbuf", bufs=1) as sb,
            tc.tile_pool(name="psum", bufs=1, space="PSUM") as pp,
            tc.tile_pool(name="psmall", bufs=2, space="PSUM") as ps,
        ):
            hdr = sb.tile([P, HC], BF, tag="hdr")
            nc.sync.dma_start(out=hdr[:], in_=hdr_d[:])
            wall = sb.tile([P, WC], BF, tag="wall")
            wkn = sb.tile([P, 2 * D], BF, tag="wkn")
            nc.scalar.dma_start(out=wall[:, W_WQ:W_WQ + 2 * D],
                                in_=wall_d[:, W_WQ:W_WQ + 2 * D])
            nc.scalar.dma_start(out=wkn[:], in_=wkn_d[:])
            nc.gpsimd.dma_start(out=wall[:, W_WV:W_WV + 2 * D],
                                in_=wall_d[:, W_WV:W_WV + 2 * D])
            nc.sync.dma_start(out=wall[:, W_WO:W_WO + 2 * D],
                              in_=wall_d[:, W_WO:W_WO + 2 * D])
            # identity built on-chip (gpsimd finishes before the DMA rings
            # even come up), keeping the DMA window for real payload
            ident = sb.tile([P, P], BF, tag="ident")
            from concourse.masks import make_identity
            make_identity(nc, ident[:])

            ej = hdr[:, C_EXPJ:C_EXPJ + NPC]

            # exp bias to f32 (activation bias operand)
            negb = sb.tile([P, 1], F32, tag="negb")
            nc.vector.tensor_copy(out=negb[:], in_=hdr[:, C_NEGB:C_NEGB + 1])

            # x_sel^T arrives pre-transposed in the hdr (host layout)
            xt = hdr[:, C_XSEL:C_XSEL + D]

            # qmT[hk, j] = (query_mine @ WQ.T)^T, computed directly in
            # transposed layout (hk on partitions) -- no PE transposes
            qmt_ps = ps.tile([P, 2 * NPC], F32, tag="ps_small")
            for u in range(2):
                for t in range(2):
                    nc.tensor.matmul(
                        out=qmt_ps[:, u * NPC:(u + 1) * NPC],
                        lhsT=wall[:, W_WQ + t * D + u * P:
                                  W_WQ + t * D + (u + 1) * P],
                        rhs=hdr[:, C_Q + t * NPC:C_Q + (t + 1) * NPC],
                        start=(t == 0), stop=(t == 1))
            # zero-pad qmT per head half so each 128-deep contraction chunk
            # only sees its own head's 64 rows: qmtp cols (u, hl, j).
            # Masked straight out of PSUM, split across vector and gpsimd.
            hmask = sb.tile([P, 2], F32, tag="hmask")
            nc.vector.tensor_copy(out=hmask[:], in_=hdr[:, C_MLO:C_MHI + 1])
            qmtp = sb.tile([P, 4 * NPC], BF, tag="qmtp")
            for u in range(2):
                for hl in range(2):
                    nc.vector.tensor_scalar(
                        out=qmtp[:, (u * 2 + hl) * NPC:
                                 (u * 2 + hl + 1) * NPC],
                        in0=qmt_ps[:, u * NPC:(u + 1) * NPC],
                        scalar1=hmask[:, hl:hl + 1], scalar2=None,
                        op0=mybir.AluOpType.mult)

            # CT[d, (h,j)] = sum_k WK[h*64+k, d] * qm[j, h*64+k]: the k-proj
            # folded into the query side, so scores are one small matmul on
            # the gathered rows instead of a full K projection.  wkn holds
            # WK natural 128-row chunks; head separation comes from qmtp.
            ct_ps = pp.tile([P, 2 * H * NPC], F32, tag="ps_ct")
            for t in range(2):
                for u in range(2):
                    nc.tensor.matmul(
                        out=ct_ps[:, t * H * NPC + u * 2 * NPC:
                                  t * H * NPC + (u + 1) * 2 * NPC],
                        lhsT=wkn[:, u * D + t * P:u * D + (t + 1) * P],
                        rhs=qmtp[:, u * 2 * NPC:(u + 1) * 2 * NPC],
                        start=True, stop=True)
            ct = sb.tile([P, 2 * H * NPC], BF, tag="ct")
            nc.vector.tensor_copy(out=ct[:], in_=ct_ps[:])

            # per-slot scores for all (head, node) pairs, then select own node
            s_ps = ps.tile([P, H * NPC], F32, tag="ps_small")
            for t in range(2):
                nc.tensor.matmul(out=s_ps[:], lhsT=xt[:, t * P:(t + 1) * P],
                                 rhs=ct[:, t * H * NPC:(t + 1) * H * NPC],
                                 start=(t == 0), stop=(t == 1))
            sm = sb.tile([P, H * NPC], F32, tag="sm")
            nc.vector.tensor_tensor(
                out=sm[:].rearrange("p (h j) -> p h j", h=H),
                in0=s_ps[:].rearrange("p (h j) -> p h j", h=H),
                in1=ej.rearrange("p (o j) -> p o j", o=1)
                    .to_broadcast([P, H, NPC]),
                op=mybir.AluOpType.mult)
            s = sb.tile([P, H], F32, tag="s")
            nc.vector.tensor_reduce(
                out=s[:], in_=sm[:].rearrange("p (h j) -> p h j", h=H),
                axis=mybir.AxisListType.X, op=mybir.AluOpType.add)

            # V projection of the gathered rows
            v_ps = pp.tile([P, D], F32, tag="ps_v")
            for t in range(2):
                nc.tensor.matmul(out=v_ps[:], lhsT=xt[:, t * P:(t + 1) * P],
                                 rhs=wall[:, W_WV + t * D:W_WV + (t + 1) * D],
                                 start=(t == 0), stop=(t == 1))
            # agg = [e-weighted v | e (| valid)]  (bf16 so the reduction
            # matmul runs at full PE rate; accumulation is f32 in PSUM)
            agg = sb.tile([P, NAGG], BF, tag="agg")
            nc.scalar.activation(out=agg[:, D:D + H], in_=s[:],
                                 func=mybir.ActivationFunctionType.Exp,
                                 bias=negb[:],
                                 scale=float(1.0 / np.sqrt(DK)))
            if guard:
                nc.vector.tensor_copy(out=agg[:, D + H:D + H + 1],
                                      in_=hdr[:, C_VALID:C_VALID + 1])
            nc.vector.tensor_tensor(
                out=agg[:, 0:D].rearrange("p (h d) -> p h d", h=H),
                in0=v_ps[:].rearrange("p (h d) -> p h d", h=H),
                in1=agg[:, D:D + H].to_broadcast([P, H, DK]),
                op=mybir.AluOpType.mult)

            # per-node reduction: [numer | denom (| count)]
            acc_ps = ps.tile([NPC, NAGG], F32, tag="ps_small")
            nc.tensor.matmul(out=acc_ps[:], lhsT=ej, rhs=agg[:],
                             start=True, stop=True)

            rec = sb.tile([NPC, H], F32, tag="rec")
            if guard:
                # guard empty nodes: denom += (count == 0)
                iszero = sb.tile([NPC, 1], F32, tag="iszero")
                nc.vector.tensor_scalar(out=iszero[:],
                                        in0=acc_ps[:, D + H:D + H + 1],
                                        scalar1=0.5, scalar2=None,
                                        op0=mybir.AluOpType.is_lt)
                den = sb.tile([NPC, H], F32, tag="den")
                nc.vector.tensor_scalar(out=den[:], in0=acc_ps[:, D:D + H],
                                        scalar1=iszero[:], scalar2=None,
                                        op0=mybir.AluOpType.add)
                nc.vector.reciprocal(out=rec[:], in_=den[:])
            else:
                nc.vector.reciprocal(out=rec[:], in_=acc_ps[:, D:D + H])
            onode = sb.tile([NPC, D], BF, tag="onode")
            nc.vector.tensor_tensor(
                out=onode[:].rearrange("p (h d) -> p h d", h=H),
                in0=acc_ps[:, 0:D].rearrange("p (h d) -> p h d", h=H),
                in1=rec[:].to_broadcast([NPC, H, DK]),
                op=mybir.AluOpType.mult)

            # r = out_node @ WO.T
            ot_ps = ps.tile([P, 2 * NPC], BF, tag="ps_small")
            for t in range(2):
                nc.tensor.transpose(out=ot_ps[:, t * NPC:(t + 1) * NPC],
                                    in_=onode[:, t * P:(t + 1) * P],
                                    identity=ident[0:NPC, 0:NPC])
            ot = sb.tile([P, 2 * NPC], BF, tag="ot")
            nc.vector.tensor_copy(out=ot[:], in_=ot_ps[:])
            # r^T directly (full-partition copies and a tiny out DMA):
            # rT[d, (t,j)] = r[j, t*128+d] = sum_u WO[t*128+d, u*128+d'] ...
            r_ps = ps.tile([P, 2 * NPC], F32, tag="ps_small")
            for t in range(2):
                for u in range(2):
                    nc.tensor.matmul(
                        out=r_ps[:, t * NPC:(t + 1) * NPC],
                        lhsT=wall[:, W_WO + u * D + t * P:
                                  W_WO + u * D + (t + 1) * P],
                        rhs=ot[:, u * NPC:(u + 1) * NPC],
                        start=(u == 0), stop=(u == 1))
            r_sb = sb.tile([P, 2 * NPC], F32, tag="r_sb")
            nc.vector.tensor_copy(out=r_sb[:], in_=r_ps[:])
            nc.sync.dma_start(out=out_d[:], in_=r_sb[:])

    nc.compile()
    return nc


def kernel(query, x, WQ, WK, WV, WO, src, dst, glob_idx):
    global last_results
    query = np.ascontiguousarray(np.asarray(query, dtype=np.float32))
    x = np.ascontiguousarray(np.asarray(x, dtype=np.float32))
    src32 = np.asarray(src, dtype=np.int32)
    dst32 = np.asarray(dst, dtype=np.int32)
    glob = np.asarray(glob_idx, dtype=np.int32)
    WQ = np.asarray(WQ, np.float32)
    WK = np.asarray(WK, np.float32)
    WV = np.asarray(WV, np.float32)
    WO = np.asarray(WO, np.float32)

    # partition (CSR-sort) edge list by dst shard (dst % 8), then dst
    shard = dst32 % NCORES
    order = np.lexsort((dst32, shard))
    s_src = src32[order]
    s_dst = dst32[order]
    s_shard = shard[order]
    shard_start = np.searchsorted(s_shard, np.arange(NCORES + 1))

    # per-global-node edge counts (for capacity + fast-path check)
    rel = dst32 < B
    gc = np.bincount(dst32[rel], minlength=B) if rel.any() else \
        np.zeros(B, np.int64)

    cap16_ok = gc.max() <= 16 if len(gc) else True
    pref_ok = all(gc[c::NCORES].sum() <= P for c in range(NCORES))
    _gc_min_zero[0] = bool(gc.min() == 0) if len(gc) else True
    fast = (np.array_equal(glob, np.arange(B, dtype=glob.dtype))
            and cap16_ok and pref_ok
            and not bool(int(os.environ.get("BASSK_FORCE_GENERAL", "0"))))

    if fast:
        res = _run_fast(query, x, s_src, s_dst, shard_start, WQ, WK, WV, WO)
    else:
        perm = np.argsort(dst32, kind="stable")
        sorted_src = np.ascontiguousarray(src32[perm])
        sorted_dst = dst32[perm]
        row_ptr = np.searchsorted(sorted_dst,
                                  np.arange(NV + 1)).astype(np.int32)
        gcnt = int((row_ptr[glob + 1] - row_ptr[glob]).max()) if len(glob) \
            else 0
        cap = 16
        while cap < gcnt:
            cap *= 2
        res = _run_general(query, x, sorted_src, row_ptr, glob, cap,
                           WQ, WK, WV, WO)
    last_results = res
    if fast:
        # per-core out is r^T [128, (t, j)]: r_c[j, t*128+d] = out[d, t*8+j]
        outs = [np.transpose(
            np.asarray(res.results[c]["out_r"]).reshape(P, 2, NPC),
            (2, 1, 0)).reshape(NPC, D) for c in range(NCORES)]
    else:
        outs = [res.results[c]["out_r"] for c in range(NCORES)]
    return np.ascontiguousarray(
        np.stack(outs, axis=1).reshape(B, D).astype(np.float32))


def _run_fast(query, x, s_src, s_dst, shard_start, WQ, WK, WV, WO):
    cap = 16
    guard = bool(_gc_min_zero[0])

    # weight wall (shared): W^T d-chunks, bf16
    wall = np.zeros((P, WC), np.float32)
    for t in range(2):
        dd = slice(t * P, (t + 1) * P)
        wall[:, W_WQ + t * D:W_WQ + (t + 1) * D] = WQ.T[dd]
        wall[:, W_WV + t * D:W_WV + (t + 1) * D] = WV.T[dd]
        wall[:, W_WO + t * D:W_WO + (t + 1) * D] = WO.T[dd]
    wall_bf = np.ascontiguousarray(wall.astype(BF16))
    # WK natural 128-row chunks side by side: wkn[p, u*D + d] = WK[u*128+p, d]
    wkn = np.ascontiguousarray(
        np.concatenate([WK[0:P, :], WK[P:2 * P, :]], axis=1).astype(BF16))

    nos = np.arange(P) // cap
    expj = np.zeros((P, NPC), np.float32)
    expj[np.arange(P), nos] = 1.0

    qT = query.T  # (D, B)
    in_maps = []
    for c in range(NCORES):
        lo, hi = int(shard_start[c]), int(shard_start[c + 1])
        sh_dst = s_dst[lo:hi]
        sh_src = s_src[lo:hi]
        n = hi - lo
        # shard-local row_ptr over my 8 nodes (c, c+8, .., c+56) + end
        my_nodes = c + NCORES * np.arange(NPC + 1)  # node c+64 bounds the end
        rp9 = np.searchsorted(sh_dst, my_nodes).astype(np.int64)
        offs_col = rp9[nos] + np.arange(P) % cap
        valid_col = (offs_col < rp9[nos + 1]).astype(np.float32)
        if n > 0:
            slot_src = np.where(offs_col < n,
                                sh_src[np.minimum(offs_col, n - 1)], 0)
        else:
            slot_src = np.zeros(P, np.int64)
        hdr = np.zeros((P, HC), np.float32)
        xs = x[slot_src]  # [128 slots, 256]; ship transposed per d-chunk
        for t in range(2):
            hdr[:, C_XSEL + t * P:C_XSEL + (t + 1) * P] = \
                xs[:, t * P:(t + 1) * P].T
        for t in range(2):
            hdr[:, C_Q + t * NPC:C_Q + (t + 1) * NPC] = \
                qT[t * P:(t + 1) * P, c::NCORES]
        hdr[:, C_EXPJ:C_EXPJ + NPC] = expj
        hdr[:, C_NEGB] = (valid_col - 1.0) * 30.0
        hdr[:, C_VALID] = valid_col
        hdr[:, C_MLO] = (np.arange(P) < DK).astype(np.float32)
        hdr[:, C_MHI] = (np.arange(P) >= DK).astype(np.float32)
        in_maps.append(dict(wall=wall_bf, wkn=wkn,
                            hdr=np.ascontiguousarray(hdr.astype(BF16))))

    key = ("fastbf", guard)
    if key not in _cache:
        _cache[key] = _build_fast(guard)
    nc = _cache[key]

    trace = bool(int(os.environ.get("BASSK_TRACE", "0")))
    return run_bass_kernel_spmd(nc, in_maps, core_ids=list(range(NCORES)),
                                trace=trace)


# ---------------------------------------------------------------------------
# general fallback (from validated v1 program)
# ---------------------------------------------------------------------------

def _expanders(cap):
    nslots = NPC * cap
    nch = nslots // P
    npc_chunk = P // cap
    expjt = np.zeros((NPC, P * nch), np.float32)
    expj = np.zeros((P, NPC * nch), np.float32)
    for k in range(nch):
        j_of_p = np.arange(P) // cap + k * npc_chunk
        expjt[j_of_p, k * P + np.arange(P)] = 1.0
        expj[np.arange(P), k * NPC + j_of_p] = 1.0
    woff = (np.arange(P) % cap).astype(np.float32)
    return expjt, expj, woff, nch


def _build_general(cap: int):
    """Build the SPMD Bass program. cap = edge slots per node (power of two,
    NPC*cap multiple of 128)."""
    nslots = NPC * cap
    n_chunks = nslots // P
    assert nslots % P == 0
    npc_chunk = P // cap  # nodes per 128-slot chunk

    nc = bacc.Bacc("TRN2", target_bir_lowering=False, debug=False,
                   num_devices=NCORES)

    # ---- DRAM I/O ----
    x_d = nc.dram_tensor("x", [NV, D], F32, kind="ExternalInput")
    srcs_d = nc.dram_tensor("srcs", [NE + cap, 1], I32, kind="ExternalInput")
    rp_d = nc.dram_tensor("row_ptr", [NV + 1, 1], I32, kind="ExternalInput")
    qy_d = nc.dram_tensor("query", [B, D], F32, kind="ExternalInput")
    wqt_d = nc.dram_tensor("wqt", [D, D], F32, kind="ExternalInput")
    wkt_d = nc.dram_tensor("wkt", [D, D], F32, kind="ExternalInput")
    wvt_d = nc.dram_tensor("wvt", [D, D], F32, kind="ExternalInput")
    wot_d = nc.dram_tensor("wot", [D, D], F32, kind="ExternalInput")
    sel_d = nc.dram_tensor("sel", [B, NPC], F32, kind="ExternalInput")
    expjt_d = nc.dram_tensor("expjt", [NPC, P * n_chunks], F32,
                             kind="ExternalInput")
    expj_d = nc.dram_tensor("expj", [P, NPC * n_chunks], F32,
                            kind="ExternalInput")
    woff_d = nc.dram_tensor("win_off", [P, 1], F32, kind="ExternalInput")
    ident_d = nc.dram_tensor("ident", [P, P], F32, kind="ExternalInput")
    mgs_d = nc.dram_tensor("my_glob_s", [NPC, 1], I32, kind="ExternalInput")
    mge_d = nc.dram_tensor("my_glob_e", [NPC, 1], I32, kind="ExternalInput")
    out_d = nc.dram_tensor("out_r", [NPC, D], F32, kind="ExternalOutput")

    with _SlimTailTileContext(nc) as tc:
        with (
            tc.tile_pool(name="const", bufs=1) as cpool,
            tc.tile_pool(name="work", bufs=1) as wpool,
            tc.tile_pool(name="psum", bufs=1, space="PSUM") as ppool,
            tc.tile_pool(name="psum_small", bufs=2, space="PSUM") as spool,
        ):
            # ---- constant / weight loads (issued early, overlap the chain) --
            qy = cpool.tile([B, D], F32, tag="qy")
            nc.sync.dma_start(out=qy[:], in_=qy_d[:])
            wq = cpool.tile([P, 2 * D], F32, tag="wq")  # [d-chunk t] at cols t*D
            wk = cpool.tile([P, 2 * D], F32, tag="wk")
            wv = cpool.tile([P, 2 * D], F32, tag="wv")
            wo = cpool.tile([P, 2 * D], F32, tag="wo")
            for t in range(2):
                nc.sync.dma_start(out=wq[:, t * D:(t + 1) * D],
                                  in_=wqt_d[t * P:(t + 1) * P, :])
                nc.sync.dma_start(out=wk[:, t * D:(t + 1) * D],
                                  in_=wkt_d[t * P:(t + 1) * P, :])
                nc.sync.dma_start(out=wv[:, t * D:(t + 1) * D],
                                  in_=wvt_d[t * P:(t + 1) * P, :])
                nc.sync.dma_start(out=wo[:, t * D:(t + 1) * D],
                                  in_=wot_d[t * P:(t + 1) * P, :])
            sel = cpool.tile([B, NPC], F32, tag="sel")
            nc.sync.dma_start(out=sel[:], in_=sel_d[:])
            expjt = cpool.tile([NPC, P * n_chunks], F32, tag="expjt")
            nc.sync.dma_start(out=expjt[:], in_=expjt_d[:])
            expj = cpool.tile([P, NPC * n_chunks], F32, tag="expj")
            nc.sync.dma_start(out=expj[:], in_=expj_d[:])
            woff = cpool.tile([P, 1], F32, tag="woff")
            nc.sync.dma_start(out=woff[:], in_=woff_d[:])
            ident = cpool.tile([P, P], F32, tag="ident")
            nc.sync.dma_start(out=ident[:], in_=ident_d[:])
            mgs = cpool.tile([NPC, 1], I32, tag="mgs")
            nc.sync.dma_start(out=mgs[:], in_=mgs_d[:])
            mge = cpool.tile([NPC, 1], I32, tag="mge")
            nc.sync.dma_start(out=mge[:], in_=mge_d[:])

            # ---- row_ptr[glob] and row_ptr[glob+1] (one indirect gather) ----
            st_i = wpool.tile([NPC, 1], I32, tag="st_i")
            nc.gpsimd.indirect_dma_start(
                out=st_i[:], out_offset=None, in_=rp_d[:],
                in_offset=IndirectOffsetOnAxis(ap=mgs[:], axis=0))
            en_i = wpool.tile([NPC, 1], I32, tag="en_i")
            nc.gpsimd.indirect_dma_start(
                out=en_i[:], out_offset=None, in_=rp_d[:],
                in_offset=IndirectOffsetOnAxis(ap=mge[:], axis=0))
            st_f = wpool.tile([NPC, 1], F32, tag="st_f")
            nc.vector.tensor_copy(out=st_f[:], in_=st_i[:])
            en_f = wpool.tile([NPC, 1], F32, tag="en_f")
            nc.vector.tensor_copy(out=en_f[:], in_=en_i[:])

            # ---- q_glob = query @ WQ.T ; q_mine = my 8 rows ----
            qyt = wpool.tile([P, 2 * B], F32, tag="qyt")  # query^T d-chunks
            for t in range(2):
                pt = spool.tile([P, B], F32, tag="ps_small")
                nc.tensor.transpose(out=pt[:], in_=qy[:, t * P:(t + 1) * P],
                                    identity=ident[:B, :B])
                nc.vector.tensor_copy(out=qyt[:, t * B:(t + 1) * B], in_=pt[:])
            qg_ps = ppool.tile([B, D], F32, tag="ps_qg")
            for t in range(2):
                nc.tensor.matmul(out=qg_ps[:], lhsT=qyt[:, t * B:(t + 1) * B],
                                 rhs=wq[:, t * D:(t + 1) * D],
                                 start=(t == 0), stop=(t == 1))
            qg = wpool.tile([B, D], F32, tag="qg")
            nc.vector.tensor_copy(out=qg[:], in_=qg_ps[:])
            qm_ps = spool.tile([NPC, D], F32, tag="ps_small")
            nc.tensor.matmul(out=qm_ps[:], lhsT=sel[:], rhs=qg[:],
                             start=True, stop=True)
            qm = wpool.tile([NPC, D], F32, tag="qm")
            nc.vector.tensor_copy(out=qm[:], in_=qm_ps[:])

            # ---- accumulator over chunks (numer | denom | count) ----
            acc = wpool.tile([NPC, D + H + 1], F32, tag="acc")

            for k in range(n_chunks):
                ejt = expjt[:, k * P:(k + 1) * P]        # [NPC, P] lhsT
                ej = expj[:, k * NPC:(k + 1) * NPC]      # [P, NPC] lhsT

                # per-slot start/end expansion
                st_ps = spool.tile([P, 1], F32, tag="ps_small")
                en_ps = spool.tile([P, 1], F32, tag="ps_small")
                nc.tensor.matmul(out=st_ps[:], lhsT=ejt, rhs=st_f[:],
                                 start=True, stop=True)
                nc.tensor.matmul(out=en_ps[:], lhsT=ejt, rhs=en_f[:],
                                 start=True, stop=True)
                offs_f = wpool.tile([P, 1], F32, tag="offs_f")
                nc.vector.tensor_add(out=offs_f[:], in0=st_ps[:], in1=woff[:])
                valid = wpool.tile([P, 1], F32, tag="valid")
                nc.vector.tensor_tensor(out=valid[:], in0=offs_f[:],
                                        in1=en_ps[:], op=mybir.AluOpType.is_lt)
                offs_i = wpool.tile([P, 1], I32, tag="offs_i")
                nc.vector.tensor_copy(out=offs_i[:], in_=offs_f[:])

                # gather src ids, then x rows
                srcv = wpool.tile([P, 1], I32, tag="srcv")
                nc.gpsimd.indirect_dma_start(
                    out=srcv[:], out_offset=None, in_=srcs_d[:],
                    in_offset=IndirectOffsetOnAxis(ap=offs_i[:], axis=0))
                xsel = wpool.tile([P, D], F32, tag="xsel")
                nc.gpsimd.indirect_dma_start(
                    out=xsel[:], out_offset=None, in_=x_d[:],
                    in_offset=IndirectOffsetOnAxis(ap=srcv[:], axis=0))

                # x_sel^T (two 128x128 transposes)
                xt = wpool.tile([P, D], F32, tag="xt")
                for t in range(2):
                    xt_ps = spool.tile([P, P], F32, tag="ps_small")
                    nc.tensor.transpose(out=xt_ps[:],
                                        in_=xsel[:, t * P:(t + 1) * P],
                                        identity=ident[:])
                    nc.vector.tensor_copy(out=xt[:, t * P:(t + 1) * P],
                                          in_=xt_ps[:])

                # K/V projections of gathered rows
                k_ps = ppool.tile([P, D], F32, tag="ps_k")
                v_ps = ppool.tile([P, D], F32, tag="ps_v")
                for t in range(2):
                    nc.tensor.matmul(out=k_ps[:], lhsT=xt[:, t * P:(t + 1) * P],
                                     rhs=wk[:, t * D:(t + 1) * D],
                                     start=(t == 0), stop=(t == 1))
                for t in range(2):
                    nc.tensor.matmul(out=v_ps[:], lhsT=xt[:, t * P:(t + 1) * P],
                                     rhs=wv[:, t * D:(t + 1) * D],
                                     start=(t == 0), stop=(t == 1))
                ksel = wpool.tile([P, D], F32, tag="ksel")
                nc.vector.tensor_copy(out=ksel[:], in_=k_ps[:])
                vsel = wpool.tile([P, D], F32, tag="vsel")
                nc.vector.tensor_copy(out=vsel[:], in_=v_ps[:])

                # qe = q row per slot
                qe_ps = ppool.tile([P, D], F32, tag="ps_qe")
                nc.tensor.matmul(out=qe_ps[:], lhsT=ejt, rhs=qm[:],
                                 start=True, stop=True)

                # scores s[p,h], e = exp(s/8) * valid
                prod = wpool.tile([P, D], F32, tag="prod")
                nc.vector.tensor_mul(out=prod[:], in0=ksel[:], in1=qe_ps[:])
                s = wpool.tile([P, H], F32, tag="s")
                nc.vector.tensor_reduce(
                    out=s[:], in_=prod[:].rearrange("p (h d) -> p h d", h=H),
                    axis=mybir.AxisListType.X, op=mybir.AluOpType.add)
                e = wpool.tile([P, H], F32, tag="e")
                nc.scalar.activation(out=e[:], in_=s[:],
                                     func=mybir.ActivationFunctionType.Exp,
                                     scale=float(1.0 / np.sqrt(DK)))
                agg = wpool.tile([P, D + H + 1], F32, tag="agg")
                nc.vector.tensor_scalar_mul(agg[:, D:D + H], e[:], valid[:])
                nc.vector.tensor_copy(out=agg[:, D + H:D + H + 1], in_=valid[:])
                # w = v * alpha-weights (per head)
                for h in range(H):
                    nc.vector.tensor_scalar_mul(
                        agg[:, h * DK:(h + 1) * DK],
                        vsel[:, h * DK:(h + 1) * DK],
                        agg[:, D + h:D + h + 1])
                # per-node reduction (numer | denom | count)
                agg_ps = spool.tile([NPC, D + H + 1], F32, tag="ps_small")
                nc.tensor.matmul(out=agg_ps[:], lhsT=ej, rhs=agg[:],
                                 start=True, stop=True)
                if n_chunks == 1:
                    nc.vector.tensor_copy(out=acc[:], in_=agg_ps[:])
                elif k == 0:
                    nc.vector.tensor_copy(out=acc[:], in_=agg_ps[:])
                else:
                    nc.vector.tensor_add(out=acc[:], in0=acc[:], in1=agg_ps[:])

            # ---- normalize: out_node = numer / max(denom, empty-guard) ----
            iszero = wpool.tile([NPC, 1], F32, tag="iszero")
            nc.vector.tensor_scalar(out=iszero[:], in0=acc[:, D + H:D + H + 1],
                                    scalar1=0.5, scalar2=None,
                                    op0=mybir.AluOpType.is_lt)
            den = wpool.tile([NPC, H], F32, tag="den")
            nc.vector.tensor_scalar(out=den[:], in0=acc[:, D:D + H],
                                    scalar1=iszero[:], scalar2=None,
                                    op0=mybir.AluOpType.add)
            rec = wpool.tile([NPC, H], F32, tag="rec")
            nc.vector.reciprocal(out=rec[:], in_=den[:])
            onode = wpool.tile([NPC, D], F32, tag="onode")
            for h in range(H):
                nc.vector.tensor_scalar_mul(
                    onode[:, h * DK:(h + 1) * DK],
                    acc[:, h * DK:(h + 1) * DK], rec[:, h:h + 1])

            # ---- r = out_node @ WO.T ----
            ot = wpool.tile([P, 2 * NPC], F32, tag="ot")
            for t in range(2):
                ot_ps = spool.tile([P, NPC], F32, tag="ps_small")
                nc.tensor.transpose(out=ot_ps[:],
                                    in_=onode[:, t * P:(t + 1) * P],
                                    identity=ident[:NPC, :NPC])
                nc.vector.tensor_copy(out=ot[:, t * NPC:(t + 1) * NPC],
                                      in_=ot_ps[:])
            r_ps = spool.tile([NPC, D], F32, tag="ps_small")
            for t in range(2):
                nc.tensor.matmul(out=r_ps[:], lhsT=ot[:, t * NPC:(t + 1) * NPC],
                                 rhs=wo[:, t * D:(t + 1) * D],
                                 start=(t == 0), stop=(t == 1))
            r_sb = wpool.tile([NPC, D], F32, tag="r_sb")
            nc.vector.tensor_copy(out=r_sb[:], in_=r_ps[:])
            nc.sync.dma_start(out=out_d[:], in_=r_sb[:])

    nc.compile()
    return nc


def _run_general(query, x, sorted_src, row_ptr, glob, cap, WQ, WK, WV, WO):
    """General fallback: arbitrary glob_idx values / larger caps."""
    expjt, expj, woff, nch = _expanders(cap)
    srcs_pad = np.concatenate(
        [sorted_src, np.zeros(cap, np.int32)]).reshape(NE + cap, 1)
    rp2 = np.ascontiguousarray(row_ptr.reshape(NV + 1, 1))
    shared = dict(
        x=x, srcs=srcs_pad, row_ptr=rp2, query=query,
        wqt=np.ascontiguousarray(WQ.T), wkt=np.ascontiguousarray(WK.T),
        wvt=np.ascontiguousarray(WV.T), wot=np.ascontiguousarray(WO.T),
        expjt=expjt, expj=expj,
        win_off=np.ascontiguousarray(woff.reshape(P, 1)),
        ident=np.eye(P, dtype=np.float32))

    in_maps = []
    for c in range(NCORES):
        mine = glob[c::NCORES]
        mgs = mine.astype(np.int32).reshape(NPC, 1)
        mge = (mine + 1).astype(np.int32).reshape(NPC, 1)
        selc = np.zeros((B, NPC), np.float32)
        selc[c + NCORES * np.arange(NPC), np.arange(NPC)] = 1.0
        in_maps.append(dict(shared, my_glob_s=mgs, my_glob_e=mge, sel=selc))

    key = ("gen", cap)
    if key not in _cache:
        _cache[key] = _build_general(cap)
    nc = _cache[key]

    trace = bool(int(os.environ.get("BASSK_TRACE", "0")))
    return run_bass_kernel_spmd(nc, in_maps, core_ids=list(range(NCORES)),
                                trace=trace)
